# revision 26
# baseline (speedup 1.0000x reference)
"""Distributed Trainium2 Bass kernel for a 16-head causal RoPE attention layer.

Problem: B=2, T=2048, D=1024, H=16, HS=64 (fp32 reference).

Sharding (8 cores): core = b*4 + g, b in {0,1} (batch), g in {0..3} (group of
4 heads).  Each core computes Q/K/V projections for its 256 head-dims, runs
causal flash-style attention for its 4 heads, and applies its 256-row slice
of Wo, producing a partial [T, D] output.  The host sums the 4 partials per
batch and adds bo.  No on-device collectives.

Pipeline (v2): the softmax exp on ScalarE is the throughput floor
(~55us/core of pure data at 1 elem/lane/cycle), so the whole kernel is
arranged to keep ScalarE maximally busy on exp and everything else
overlapped under it:
  - scores for a HEAD PAIR share one [128, 2x512] PSUM tile (head A cols
    0:512, head B 512:1024) written by two row-group-concurrent 64-row
    matmuls (tile_position (0,0)/(64,0) auto-derived), so each exp op
    covers 1024 cols -> half the ACT per-op pipeline overhead.
  - attention is q-quarter major (512 q cols per step) so Y for both
    heads fits one [128, 2x512] PSUM tile; flash accumulation over k
    tiles; denominators via 64 ones-columns in the V stationary.
  - q/k biases are folded into the projection matmuls as a 9th
    contraction row (ones moving row), evictions are pure DVE casts.
  - RoPE runs in 512-col chunks (cast -> 4 DVE partition-shift copies ->
    2 DVE muls -> GpSimd add) so attention starts early.
  - normalize: 1/r = exp(-ln r) on ScalarE over both heads at once.
  - diagonal 128x128 blocks: exp trimmed to the valid cols, triangular
    mask applied multiplicatively on DVE.
PSUM: scores 2 bufs x 2 banks + Y 1 buf x 2 banks + proj/outproj ring
2 bufs x 1 bank = 8 banks exactly.
"""

import numpy as np
import ml_dtypes

import concourse.bass as bass
import concourse.mybir as mybir
import concourse.tile as tile
from concourse.bass_utils import run_bass_kernel_spmd

BF16 = mybir.dt.bfloat16
F32 = mybir.dt.float32

B, T, D = 2, 2048, 1024
H, HS = 16, 64
THETA = 10000.0
NCORES = 8
HG = 4            # heads per core
HD = HG * HS      # head dims per core = 256
SCALE = 1.0 / 8.0  # 1/sqrt(HS)
NEG = -1.0e5       # additive mask for padded keys (exp underflows to 0)

_NC = None


_SELF_SEM = {
    "EngineType.Activation": "Activation_",
    "EngineType.DVE": "DVE_",
    "EngineType.PE": "PE_",
    "EngineType.Pool": "Pool_",
}


def _split_multi_waits(nc):
    """walrus codegen accepts at most ONE semaphore wait per engine
    instruction (the 64B ISA structs have a single EVENTS slot); Tile's
    scheduler freely emits several.  Hoist all but the last wait of each
    instruction onto inserted same-engine EventSemaphore (poll_sem) ops,
    which preserves semantics exactly (engines execute sequentially).

    Additionally drop ge-waits on the instruction's OWN engine semaphore
    for compute engines: those guard WAW/WAR against earlier instructions
    of the same in-order engine, which program order already guarantees
    (each op's writes drain before the next op's visible effects).  Tile
    emits one before nearly every exp in the attention loop; at ~100ns of
    sequencer dispatch each they are pure overhead."""
    def _names(args):
        out = set()
        for a in args:
            for attr in ("memref", "name"):
                v = getattr(a, attr, None)
                if isinstance(v, str):
                    out.add(v.removesuffix("_set"))
            t = getattr(a, "tensor", None)
            if t is not None and isinstance(getattr(t, "name", None), str):
                out.add(t.name)
        return out

    # per-engine written/read tensor sets: an op READING an own-engine-
    # written tensor (RAW) or WRITING an own-engine-read tensor (WAR) has a
    # genuine same-engine hazard through the deep pipeline, so its self-wait
    # must survive; pure WAW through the in-order write port is safe.
    eng_written = {}
    eng_read = {}
    _COMPUTE = {"InstActivation", "InstTensorTensor", "InstTensorCopy",
                "InstMatmult", "InstLdweights", "InstMemset",
                "InstTensorScalarPtr", "InstTensorReduce"}
    for f in nc.m.functions:
        for blk in f.blocks:
            for inst in blk.instructions:
                if type(inst).__name__ in _COMPUTE:
                    e = str(inst.engine)
                    eng_written.setdefault(e, set()).update(_names(inst.outs))
                    eng_read.setdefault(e, set()).update(_names(inst.ins))

    n = 0
    for f in nc.m.functions:
        for blk in f.blocks:
            il = blk.instructions
            i = 0
            while i < len(il):
                inst = il[i]
                si = inst.sync_info
                if si is None or not si.on_wait:
                    i += 1
                    continue
                waits = list(si.on_wait)
                eng = str(inst.engine)
                selfpfx = _SELF_SEM.get(eng)
                if (selfpfx is not None
                        and type(inst).__name__ == "InstActivation"
                        and not (_names(inst.ins) & eng_written.get(eng, set()))
                        and not (_names(inst.outs) & eng_read.get(eng, set()))):
                    kept = [w for w in waits
                            if not (w.wait_mode == "sem-ge-imm"
                                    and w.ant_name.startswith(selfpfx))]
                    if len(kept) != len(waits):
                        waits = kept
                        inst.sync_info = mybir.SyncInfo(
                            on_wait=waits, on_update=list(si.on_update))
                if len(waits) > 1:
                    for w in waits[:-1]:
                        es = mybir.InstEventSemaphore(name=f"I-wsplit-{n}")
                        n += 1
                        es.engine = inst.engine
                        es.sync_info = mybir.SyncInfo(on_wait=[w], on_update=[])
                        nc.register_instruction(es)
                        il.insert(i, es)
                        i += 1
                    inst.sync_info = mybir.SyncInfo(
                        on_wait=[waits[-1]], on_update=list(si.on_update))
                i += 1
    return n


def _dedup_ldweights(nc):
    """bass emits one InstLdweights per InstMatmult.  When a later
    InstLdweights loads the IDENTICAL weights AP that is already resident
    in the PE array (no other InstLdweights in between), the reload is
    redundant: MATMUL does not self-load for 16-bit dtypes.  Delete it,
    folding its waits into the following matmult (whose multi-waits are
    then legalized by _split_multi_waits)."""
    def fp(inst):
        a = inst.ins[0]
        return (a.memref, a.offset, str(a.ap), str(a.dtype))

    n = 0
    for f in nc.m.functions:
        for blk in f.blocks:
            il = blk.instructions
            last = None
            i = 0
            while i < len(il):
                inst = il[i]
                tn = type(inst).__name__
                if tn == "InstLdweights":
                    cur = fp(inst)
                    si = inst.sync_info
                    if cur == last and (si is None or not si.on_update):
                        waits = list(si.on_wait) if si is not None else []
                        if waits:
                            j = i + 1
                            while (j < len(il)
                                   and type(il[j]).__name__ != "InstMatmult"):
                                j += 1
                            if j == len(il):
                                i += 1
                                continue
                            mm = il[j]
                            msi = mm.sync_info
                            mw = list(msi.on_wait) if msi is not None else []
                            mu = list(msi.on_update) if msi is not None else []
                            mm.sync_info = mybir.SyncInfo(
                                on_wait=waits + mw, on_update=mu)
                        del il[i]
                        n += 1
                        continue
                    last = cur
                i += 1
    return n


def build_nc():
    nc = bass.Bass()

    xT = nc.declare_dram_parameter("xT", [D, T], BF16, isOutput=False)
    wq = nc.declare_dram_parameter("wq", [D, HD], BF16, isOutput=False)
    wk = nc.declare_dram_parameter("wk", [D, HD], BF16, isOutput=False)
    wv = nc.declare_dram_parameter("wv", [D, HD], BF16, isOutput=False)
    wo = nc.declare_dram_parameter("wo", [HD, D], BF16, isOutput=False)
    # [bq(256) | bk(256)] as a single stationary bias row
    brow = nc.declare_dram_parameter("brow", [1, 2 * HD], BF16, isOutput=False)
    bv = nc.declare_dram_parameter("bv", [1, HD], F32, isOutput=False)
    cos2 = nc.declare_dram_parameter("cos2", [128, T], BF16, isOutput=False)
    sin2 = nc.declare_dram_parameter("sin2", [128, T], BF16, isOutput=False)
    tri2 = nc.declare_dram_parameter("tri2", [128, 256], BF16, isOutput=False)
    kb = nc.declare_dram_parameter("kb", [T], F32, isOutput=False)
    out = nc.declare_dram_parameter("out", [T, D], BF16, isOutput=True)

    NT = T // 512   # 4 T-ranges for projections
    NK = T // 128   # 16 key tiles

    with tile.TileContext(nc) as tc:
        with (
            tc.tile_pool(name="const", bufs=1) as cpool,
            tc.tile_pool(name="xw", bufs=1) as xwpool,
            tc.tile_pool(name="qk", bufs=1) as qkpool,
            tc.tile_pool(name="raw", bufs=3) as rawpool,
            tc.tile_pool(name="p", bufs=6) as ppool,
            tc.tile_pool(name="rec", bufs=2) as rpool,
            tc.tile_pool(name="ev", bufs=3) as evpool,
            tc.tile_pool(name="psP", bufs=2, space="PSUM") as psP,
            tc.tile_pool(name="psS", bufs=2, space="PSUM") as psS,
            tc.tile_pool(name="psY", bufs=1, space="PSUM") as psY,
        ):
            # ---- constant / weight loads ----
            # order matters: wq + the first xT column-chunks gate the first
            # matmul groups, so issue them first; xT is loaded in 512-col
            # chunks so projection groups start after ~1MB, not 4MB.
            wq_sb = xwpool.tile([128, 8, HD], BF16, tag="wq")
            wk_sb = xwpool.tile([128, 8, HD], BF16, tag="wk")
            wv_sb = xwpool.tile([128, 8, HD], BF16, tag="wv")
            wo_sb = xwpool.tile([128, 2, D], BF16, tag="wo")
            brow_sb = cpool.tile([1, 2 * HD], BF16, tag="brow")
            ones_sb = cpool.tile([1, 512], BF16, tag="ones")
            bv_sb = cpool.tile([128, HD], F32, tag="bv")
            wq_r = wq.ap().rearrange("(c p) n -> p c n", p=128)
            wk_r = wk.ap().rearrange("(c p) n -> p c n", p=128)
            # pair-0 (c2=0) weight halves first: the first attention quarter
            # needs only these
            nc.sync.dma_start(wq_sb[:, :, 0:128], wq_r[:, :, 0:128])
            nc.sync.dma_start(brow_sb[:], brow.ap())
            nc.vector.memset(ones_sb[:], 1.0)
            # dependency-free warmup matmuls: ~5us of PE activity while the
            # input DMAs stream, so the HAM clock gate is already at 8/8
            # (2.4GHz) when the first real projection group issues
            warm_ps = psP.tile([128, 512], F32, tag="pp", name="warm")
            for i in range(12):
                nc.tensor.matmul(
                    warm_ps[:], ones_sb[0:1, 0:128], ones_sb[0:1, :],
                    start=(i == 0), stop=(i == 11),
                    skip_group_check=True,
                )

            xts = []
            for dc in range(8):
                xt = xwpool.tile([128, T], BF16, tag=f"xt{dc}", name=f"xt{dc}")
                xts.append(xt)

            def load_xt_tr(tr):
                for dc in range(8):
                    nc.sync.dma_start(
                        xts[dc][:, tr * 512:(tr + 1) * 512],
                        xT[dc * 128:(dc + 1) * 128, tr * 512:(tr + 1) * 512],
                    )

            cos_sb = cpool.tile([128, T], BF16, tag="cos")
            sin_sb = cpool.tile([128, T], BF16, tag="sin")
            tri2_sb = cpool.tile([128, 2, 128], BF16, tag="tri2")
            kb_sb = cpool.tile([128, NK], F32, tag="kb")
            # everything the first attention quarter needs (q/k tr0 chunks,
            # rope tables, wv, exp consts) comes before the bulk of xT so
            # the lead-in isn't DMA-gated
            load_xt_tr(0)
            nc.sync.dma_start(wk_sb[:, :, 0:128], wk_r[:, :, 0:128])
            nc.sync.dma_start(cos_sb[:, 0:512], cos2[:, 0:512])
            nc.sync.dma_start(sin_sb[:, 0:512], sin2[:, 0:512])
            nc.sync.dma_start(wv_sb[:], wv.ap().rearrange("(c p) n -> p c n", p=128))
            nc.sync.dma_start(
                tri2_sb[:], tri2.ap().rearrange("p (two q) -> p two q", two=2))
            nc.sync.dma_start(kb_sb[:], kb.ap().rearrange("(t p) -> p t", p=128))
            nc.sync.dma_start(bv_sb[:], bv.ap().to_broadcast((128, HD)))
            load_xt_tr(1)
            nc.sync.dma_start(cos_sb[:, 512:T], cos2[:, 512:T])
            nc.sync.dma_start(sin_sb[:, 512:T], sin2[:, 512:T])
            load_xt_tr(2)
            load_xt_tr(3)
            nc.sync.dma_start(wq_sb[:, :, 128:256], wq_r[:, :, 128:256])
            nc.sync.dma_start(wk_sb[:, :, 128:256], wk_r[:, :, 128:256])
            nc.sync.dma_start(wo_sb[:], wo.ap().rearrange("(c p) n -> p c n", p=128))

            # persistent [128, T] tiles: 2 heads each (rows 0:64 / 64:128)
            qT = [qkpool.tile([128, T], BF16, tag=f"qT{c}", name=f"qT{c}") for c in range(2)]
            kT = [qkpool.tile([128, T], BF16, tag=f"kT{c}", name=f"kT{c}") for c in range(2)]
            yT = [qkpool.tile([128, T], BF16, tag=f"yT{c}", name=f"yT{c}") for c in range(2)]

            # ---- Q^T / K^T projection + RoPE, one 512-col chunk ----
            # ti: 0 = q, 1 = k (selects bias row slice)
            def proj_qk_chunk(ti, wsb, c2, fin, tr):
                lo, hi = tr * 512, (tr + 1) * 512
                ps = psP.tile([128, 512], F32, tag="pp")
                for dc in range(8):
                    nc.tensor.matmul(
                        ps[:],
                        wsb[:, dc, c2 * 128:(c2 + 1) * 128],
                        xts[dc][:, lo:hi],
                        start=(dc == 0),
                        stop=False,
                    )
                # bias via rank-1 update: ones row x bias row
                nc.tensor.matmul(
                    ps[:],
                    brow_sb[0:1, ti * HD + c2 * 128: ti * HD + (c2 + 1) * 128],
                    ones_sb[0:1, :],
                    start=False, stop=True,
                )
                raw = rawpool.tile([128, 512], BF16, tag="raw")
                nc.vector.tensor_copy(raw[:], ps[:])
                # RoPE: fin = raw*cos + rot(raw)*sin_signed
                f = fin
                for (do, di) in ((0, 32), (32, 0), (64, 96), (96, 64)):
                    nc.vector.tensor_copy(f[do:do + 32, lo:hi], raw[di:di + 32, :])
                nc.vector.tensor_mul(f[:, lo:hi], f[:, lo:hi], sin_sb[:, lo:hi])
                nc.vector.tensor_mul(raw[:], raw[:], cos_sb[:, lo:hi])
                # final add on GpSimd (idle) to unload DVE
                nc.gpsimd.tensor_add(f[:, lo:hi], f[:, lo:hi], raw[:])

            # same projection for TWO 512-col chunks, dc-major over two psum
            # tiles: each stationary slice serves both chunks back-to-back so
            # _dedup_ldweights deletes every second (identical) weight load.
            def proj_qk_pair(ti, wsb, c2, fin, trp):
                trs = (2 * trp, 2 * trp + 1)
                pss = [psP.tile([128, 512], F32, tag="pp", name=f"pp{t}")
                       for t in range(2)]
                for dc in range(8):
                    for t in range(2):
                        lo = trs[t] * 512
                        nc.tensor.matmul(
                            pss[t][:],
                            wsb[:, dc, c2 * 128:(c2 + 1) * 128],
                            xts[dc][:, lo:lo + 512],
                            start=(dc == 0),
                            stop=False,
                            skip_group_check=True,
                        )
                for t in range(2):
                    nc.tensor.matmul(
                        pss[t][:],
                        brow_sb[0:1, ti * HD + c2 * 128: ti * HD + (c2 + 1) * 128],
                        ones_sb[0:1, :],
                        start=False, stop=True,
                        skip_group_check=True,
                    )
                for t in range(2):
                    lo = trs[t] * 512
                    hi = lo + 512
                    raw = rawpool.tile([128, 512], BF16, tag="raw")
                    nc.vector.tensor_copy(raw[:], pss[t][:])
                    f = fin
                    for (do, di) in ((0, 32), (32, 0), (64, 96), (96, 64)):
                        nc.vector.tensor_copy(f[do:do + 32, lo:hi], raw[di:di + 32, :])
                    nc.vector.tensor_mul(f[:, lo:hi], f[:, lo:hi], sin_sb[:, lo:hi])
                    nc.vector.tensor_mul(raw[:], raw[:], cos_sb[:, lo:hi])
                    nc.gpsimd.tensor_add(f[:, lo:hi], f[:, lo:hi], raw[:])

            # ---- V projection (normal layout, with bias and ones blocks) ----
            vts = [None] * NK

            def proj_v(kt):
                ps = psP.tile([128, HD], F32, tag="pp")
                for dc in range(8):
                    nc.tensor.matmul(
                        ps[:],
                        xts[dc][:, kt * 128:(kt + 1) * 128],
                        wv_sb[:, dc, :],
                        start=(dc == 0),
                        stop=(dc == 7),
                    )
                vt = xwpool.tile([128, HG, 128], BF16, tag=f"v{kt}", name=f"v{kt}")
                nc.vector.tensor_add(
                    vt[:, :, 0:64],
                    ps[:].rearrange("p (h d) -> p h d", h=HG),
                    bv_sb[:].rearrange("p (h d) -> p h d", h=HG),
                )
                nc.vector.memset(vt[:, :, 64:128], 1.0)
                vts[kt] = vt

            # ---- attention for head pair c2, one q-quarter (512 cols) ----
            # scores/probs/Y for both heads live side by side in one
            # [128, 2, 512] tile: [:, 0, :] = head 2*c2, [:, 1, :] = 2*c2+1.
            def attn_quarter(c2, qq):
                qlo = qq * 512
                last = 4 * qq + 3
                y = psY.tile([128, 2, 512], F32, tag="y", name=f"y{c2}_{qq}")
                for kt in range(last + 1):
                    j = kt - 4 * qq
                    c = j * 128 if j >= 0 else 0   # first valid col (diag trim)
                    ksl = slice(kt * 128, (kt + 1) * 128)
                    qsl = slice(qlo + c, qlo + 512)
                    s = psS.tile([128, 2, 512], F32, tag="s")
                    # two row-group-concurrent 64-row score matmuls
                    nc.tensor.matmul(
                        s[:, 0, c:], kT[c2][0:64, ksl], qT[c2][0:64, qsl],
                        start=True, stop=True,
                    )
                    nc.tensor.matmul(
                        s[:, 1, c:], kT[c2][64:128, ksl], qT[c2][64:128, qsl],
                        start=True, stop=True,
                    )
                    p = ppool.tile([128, 2, 512], BF16, tag="p")
                    nc.scalar.activation(
                        p[:, :, c:], s[:, :, c:],
                        mybir.ActivationFunctionType.Exp,
                        bias=kb_sb[:, kt:kt + 1], scale=SCALE,
                    )
                    if j >= 0:
                        # diagonal 128x128 blocks of both heads: tri mask
                        nc.vector.tensor_mul(
                            p[:, :, c:c + 128], p[:, :, c:c + 128], tri2_sb[:]
                        )
                    for h in (0, 1):
                        nc.tensor.matmul(
                            y[:, h, c:],
                            vts[kt][:, 2 * c2 + h, :],
                            p[:, h, c:],
                            start=(kt == 0),
                            stop=(kt == last),
                            skip_group_check=True,
                        )
                # normalize both heads at once: 1/r = exp(-ln r)
                lnr = rpool.tile([64, 2, 512], F32, tag="lnr")
                rec = rpool.tile([64, 2, 512], F32, tag="rec")
                nc.scalar.activation(
                    lnr[:], y[64:128, :, :], mybir.ActivationFunctionType.Ln)
                nc.scalar.activation(
                    rec[:], lnr[:], mybir.ActivationFunctionType.Exp,
                    scale=-1.0)
                nc.vector.tensor_mul(
                    yT[c2][0:64, qlo:qlo + 512], y[0:64, 0, :], rec[:, 0, :])
                nc.vector.tensor_mul(
                    yT[c2][64:128, qlo:qlo + 512], y[0:64, 1, :], rec[:, 1, :])

            def outproj_quarter(qq):
                # partial out for T-tiles of this quarter; host adds bo+reduces
                # c2-outer / dr-inner: each yT stationary slice serves both wo
                # column halves (second load deleted by _dedup_ldweights)
                for tt in range(4 * qq, 4 * qq + 4):
                    pss = [psP.tile([128, 512], F32, tag="pp", name=f"po{t}")
                           for t in range(2)]
                    for c2 in range(2):
                        for dr in range(2):
                            nc.tensor.matmul(
                                pss[dr][:],
                                yT[c2][:, tt * 128:(tt + 1) * 128],
                                wo_sb[:, c2, dr * 512:(dr + 1) * 512],
                                start=(c2 == 0),
                                stop=(c2 == 1),
                                skip_group_check=True,
                            )
                    for dr in range(2):
                        ev = evpool.tile([128, 512], BF16, tag="ev")
                        nc.vector.tensor_copy(ev[:], pss[dr][:])
                        nc.sync.dma_start(
                            out[tt * 128:(tt + 1) * 128, dr * 512:(dr + 1) * 512],
                            ev[:],
                        )

            # ---- emission order == scheduler priority ----
            # quarter qq of pair 0 needs exactly q/k chunks tr<=qq and
            # V tiles kt<=4qq+3, so interleave per quarter: attention
            # starts right after the first 1.5MB of DMA instead of after
            # the whole projection phase.
            # quarters 0/1 use single chunks (tr0 must not wait on tr1's xT
            # DMA in the lead-in); later projections go in reuse-pairs
            for qq in range(2):
                proj_qk_chunk(0, wq_sb, 0, qT[0], qq)
                proj_qk_chunk(1, wk_sb, 0, kT[0], qq)
                for kt in range(4 * qq, 4 * qq + 4):
                    proj_v(kt)
                attn_quarter(0, qq)
                if qq == 1:
                    # V 8-11 early (inputs ready): spreads the PE crunch of
                    # the attn(0,2)/(0,3) region into the qq1 window
                    for kt in range(8, 12):
                        proj_v(kt)
            proj_qk_pair(0, wq_sb, 0, qT[0], 1)
            proj_qk_pair(1, wk_sb, 0, kT[0], 1)
            attn_quarter(0, 2)
            # V 12-15 before the pair-1 projections: attn(0,3) needs them
            # and the psP ring serves groups in emission order
            for kt in range(12, 16):
                proj_v(kt)
            proj_qk_pair(0, wq_sb, 1, qT[1], 0)
            proj_qk_pair(1, wk_sb, 1, kT[1], 0)
            attn_quarter(0, 3)
            proj_qk_pair(0, wq_sb, 1, qT[1], 1)
            proj_qk_pair(1, wk_sb, 1, kT[1], 1)
            # out-projection emitted one quarter late: its dense matmuls fill
            # the attention exp-wait gaps instead of outranking the next
            # quarter's score matmuls (it shares no psum ring with attention)
            for qq in range(4):
                attn_quarter(1, qq)
                if qq >= 1:
                    outproj_quarter(qq - 1)
            outproj_quarter(3)
    nd = _dedup_ldweights(nc)
    _split_multi_waits(nc)
    assert nd > 0, f"expected ldweights dedup to fire, got {nd}"
    return nc


def _rope_tables():
    inv_freq = 1.0 / (THETA ** (np.arange(0, HS, 2, dtype=np.float64) / HS))  # [32]
    t = np.arange(T, dtype=np.float64)
    fr = t[:, None] * inv_freq[None, :]          # [T, 32]
    emb = np.concatenate([fr, fr], axis=1)       # [T, 64]
    cos = np.cos(emb).T.astype(np.float32)       # [64, T]
    sin = np.sin(emb).T.astype(np.float32)       # [64, T]
    sin_signed = sin.copy()
    sin_signed[0:32] = -sin_signed[0:32]
    cos2 = np.concatenate([cos, cos], axis=0)            # [128, T]
    sin2 = np.concatenate([sin_signed, sin_signed], 0)   # [128, T]
    return cos2.astype(ml_dtypes.bfloat16), sin2.astype(ml_dtypes.bfloat16)


def _in_maps(x, attention_mask, Wq, bqv, Wk, bkv, Wv, bvv, Wo):
    cos2, sin2 = _rope_tables()
    tri = np.triu(np.ones((128, 128), np.float32))
    tri2 = np.concatenate([tri, tri], axis=1).astype(ml_dtypes.bfloat16)
    bf = ml_dtypes.bfloat16
    xTs = [np.ascontiguousarray(x[b].T).astype(bf) for b in range(B)]
    kbs = [
        np.where(attention_mask[b] != 0, 0.0, NEG).astype(np.float32)
        for b in range(B)
    ]
    maps = []
    for core in range(NCORES):
        b, g = core // 4, core % 4
        sl = slice(g * HD, (g + 1) * HD)
        brow = np.concatenate([bqv[sl], bkv[sl]]).reshape(1, 2 * HD)
        maps.append({
            "xT": xTs[b],
            "wq": np.ascontiguousarray(Wq[:, sl]).astype(bf),
            "wk": np.ascontiguousarray(Wk[:, sl]).astype(bf),
            "wv": np.ascontiguousarray(Wv[:, sl]).astype(bf),
            "wo": np.ascontiguousarray(Wo[sl, :]).astype(bf),
            "brow": brow.astype(bf),
            "bv": bvv[sl].astype(np.float32).reshape(1, HD),
            "cos2": cos2,
            "sin2": sin2,
            "tri2": tri2,
            "kb": kbs[b],
        })
    return maps


def _run(inputs, trace=False):
    global _NC
    if _NC is None:
        _NC = build_nc()
    maps = _in_maps(
        np.asarray(inputs["x"]), np.asarray(inputs["attention_mask"]),
        np.asarray(inputs["Wq"]), np.asarray(inputs["bq"]),
        np.asarray(inputs["Wk"]), np.asarray(inputs["bk"]),
        np.asarray(inputs["Wv"]), np.asarray(inputs["bv"]),
        np.asarray(inputs["Wo"]),
    )
    res = run_bass_kernel_spmd(_NC, maps, core_ids=list(range(NCORES)), trace=trace)
    bo = np.asarray(inputs["bo"], np.float32)
    outs = []
    for b in range(B):
        acc = np.zeros((T, D), np.float32)
        for g in range(4):
            acc += np.asarray(res.results[b * 4 + g]["out"], np.float32)
        outs.append(acc + bo[None, :])
    return np.stack(outs, axis=0), res


def kernel(**inputs):
    out, _ = _run(inputs, trace=False)
    return out


# revision 28
# speedup vs baseline: 1.1464x; 1.1464x over previous
"""Distributed Trainium2 Bass kernel for a 16-head causal RoPE attention layer.

Problem: B=2, T=2048, D=1024, H=16, HS=64 (fp32 reference).

Sharding (8 cores): core = b*4 + g, b in {0,1} (batch), g in {0..3} (group of
4 heads).  Each core computes Q/K/V projections for its 256 head-dims, runs
causal flash-style attention for its 4 heads, and applies its 256-row slice
of Wo, producing a partial [T, D] output.  The host sums the 4 partials per
batch and adds bo.  No on-device collectives.

Pipeline (v2): the softmax exp on ScalarE is the throughput floor
(~55us/core of pure data at 1 elem/lane/cycle), so the whole kernel is
arranged to keep ScalarE maximally busy on exp and everything else
overlapped under it:
  - scores for a HEAD PAIR share one [128, 2x512] PSUM tile (head A cols
    0:512, head B 512:1024) written by two row-group-concurrent 64-row
    matmuls (tile_position (0,0)/(64,0) auto-derived), so each exp op
    covers 1024 cols -> half the ACT per-op pipeline overhead.
  - attention is q-quarter major (512 q cols per step) so Y for both
    heads fits one [128, 2x512] PSUM tile; flash accumulation over k
    tiles; denominators via 64 ones-columns in the V stationary.
  - q/k biases are folded into the projection matmuls as a 9th
    contraction row (ones moving row), evictions are pure DVE casts.
  - RoPE runs in 512-col chunks (cast -> 4 DVE partition-shift copies ->
    2 DVE muls -> GpSimd add) so attention starts early.
  - normalize: 1/r = exp(-ln r) on ScalarE over both heads at once.
  - diagonal 128x128 blocks: exp trimmed to the valid cols, triangular
    mask applied multiplicatively on DVE.
PSUM: scores 2 bufs x 2 banks + Y 1 buf x 2 banks + proj/outproj ring
2 bufs x 1 bank = 8 banks exactly.
"""

import numpy as np
import ml_dtypes

import concourse.bass as bass
import concourse.mybir as mybir
import concourse.tile as tile
from concourse.bass_utils import run_bass_kernel_spmd

BF16 = mybir.dt.bfloat16
F32 = mybir.dt.float32

B, T, D = 2, 2048, 1024
H, HS = 16, 64
THETA = 10000.0
NCORES = 8
HG = 4            # heads per core
HD = HG * HS      # head dims per core = 256
SCALE = 1.0 / 8.0  # 1/sqrt(HS)
NEG = -1.0e5       # additive mask for padded keys (exp underflows to 0)

_NC = None


_SELF_SEM = {
    "EngineType.Activation": "Activation_",
    "EngineType.DVE": "DVE_",
    "EngineType.PE": "PE_",
    "EngineType.Pool": "Pool_",
}


def _split_multi_waits(nc):
    """walrus codegen accepts at most ONE semaphore wait per engine
    instruction (the 64B ISA structs have a single EVENTS slot); Tile's
    scheduler freely emits several.  Hoist all but the last wait of each
    instruction onto inserted same-engine EventSemaphore (poll_sem) ops,
    which preserves semantics exactly (engines execute sequentially).

    Additionally drop ge-waits on the instruction's OWN engine semaphore
    for compute engines: those guard WAW/WAR against earlier instructions
    of the same in-order engine, which program order already guarantees
    (each op's writes drain before the next op's visible effects).  Tile
    emits one before nearly every exp in the attention loop; at ~100ns of
    sequencer dispatch each they are pure overhead."""
    def _names(args):
        out = set()
        for a in args:
            for attr in ("memref", "name"):
                v = getattr(a, attr, None)
                if isinstance(v, str):
                    out.add(v.removesuffix("_set"))
            t = getattr(a, "tensor", None)
            if t is not None and isinstance(getattr(t, "name", None), str):
                out.add(t.name)
        return out

    # per-engine written/read tensor sets: an op READING an own-engine-
    # written tensor (RAW) or WRITING an own-engine-read tensor (WAR) has a
    # genuine same-engine hazard through the deep pipeline, so its self-wait
    # must survive; pure WAW through the in-order write port is safe.
    eng_written = {}
    eng_read = {}
    _COMPUTE = {"InstActivation", "InstTensorTensor", "InstTensorCopy",
                "InstMatmult", "InstLdweights", "InstMemset",
                "InstTensorScalarPtr", "InstTensorReduce"}
    for f in nc.m.functions:
        for blk in f.blocks:
            for inst in blk.instructions:
                if type(inst).__name__ in _COMPUTE:
                    e = str(inst.engine)
                    eng_written.setdefault(e, set()).update(_names(inst.outs))
                    eng_read.setdefault(e, set()).update(_names(inst.ins))

    n = 0
    for f in nc.m.functions:
        for blk in f.blocks:
            il = blk.instructions
            i = 0
            while i < len(il):
                inst = il[i]
                si = inst.sync_info
                if si is None or not si.on_wait:
                    i += 1
                    continue
                waits = list(si.on_wait)
                eng = str(inst.engine)
                selfpfx = _SELF_SEM.get(eng)
                if (selfpfx is not None
                        and type(inst).__name__ == "InstActivation"
                        and not (_names(inst.ins) & eng_written.get(eng, set()))
                        and not (_names(inst.outs) & eng_read.get(eng, set()))):
                    kept = [w for w in waits
                            if not (w.wait_mode == "sem-ge-imm"
                                    and w.ant_name.startswith(selfpfx))]
                    if len(kept) != len(waits):
                        waits = kept
                        inst.sync_info = mybir.SyncInfo(
                            on_wait=waits, on_update=list(si.on_update))
                if len(waits) > 1:
                    for w in waits[:-1]:
                        es = mybir.InstEventSemaphore(name=f"I-wsplit-{n}")
                        n += 1
                        es.engine = inst.engine
                        es.sync_info = mybir.SyncInfo(on_wait=[w], on_update=[])
                        nc.register_instruction(es)
                        il.insert(i, es)
                        i += 1
                    inst.sync_info = mybir.SyncInfo(
                        on_wait=[waits[-1]], on_update=list(si.on_update))
                i += 1
    return n


def _dedup_ldweights(nc):
    """bass emits one InstLdweights per InstMatmult.  When a later
    InstLdweights loads the IDENTICAL weights AP that is already resident
    in the PE array (no other InstLdweights in between), the reload is
    redundant: MATMUL does not self-load for 16-bit dtypes.  Delete it,
    folding its waits into the following matmult (whose multi-waits are
    then legalized by _split_multi_waits)."""
    def fp(inst):
        a = inst.ins[0]
        return (a.memref, a.offset, str(a.ap), str(a.dtype))

    n = 0
    for f in nc.m.functions:
        for blk in f.blocks:
            il = blk.instructions
            last = None
            i = 0
            while i < len(il):
                inst = il[i]
                tn = type(inst).__name__
                if tn == "InstLdweights":
                    cur = fp(inst)
                    si = inst.sync_info
                    if cur == last and (si is None or not si.on_update):
                        waits = list(si.on_wait) if si is not None else []
                        if waits:
                            j = i + 1
                            while (j < len(il)
                                   and type(il[j]).__name__ != "InstMatmult"):
                                j += 1
                            if j == len(il):
                                i += 1
                                continue
                            mm = il[j]
                            msi = mm.sync_info
                            mw = list(msi.on_wait) if msi is not None else []
                            mu = list(msi.on_update) if msi is not None else []
                            mm.sync_info = mybir.SyncInfo(
                                on_wait=waits + mw, on_update=mu)
                        del il[i]
                        n += 1
                        continue
                    last = cur
                i += 1
    return n


def build_nc():
    nc = bass.Bass()

    xT = nc.declare_dram_parameter("xT", [D, T], BF16, isOutput=False)
    wq = nc.declare_dram_parameter("wq", [D, HD], BF16, isOutput=False)
    wk = nc.declare_dram_parameter("wk", [D, HD], BF16, isOutput=False)
    wv = nc.declare_dram_parameter("wv", [D, HD], BF16, isOutput=False)
    wo = nc.declare_dram_parameter("wo", [HD, D], BF16, isOutput=False)
    # [bq(256) | bk(256)] as a single stationary bias row
    brow = nc.declare_dram_parameter("brow", [1, 2 * HD], BF16, isOutput=False)
    bv = nc.declare_dram_parameter("bv", [1, HD], F32, isOutput=False)
    cos2 = nc.declare_dram_parameter("cos2", [128, T], BF16, isOutput=False)
    sin2 = nc.declare_dram_parameter("sin2", [128, T], BF16, isOutput=False)
    tri2 = nc.declare_dram_parameter("tri2", [128, 256], BF16, isOutput=False)
    kb = nc.declare_dram_parameter("kb", [T], F32, isOutput=False)
    out = nc.declare_dram_parameter("out", [T, D], BF16, isOutput=True)

    NT = T // 512   # 4 T-ranges for projections
    NK = T // 128   # 16 key tiles

    with tile.TileContext(nc) as tc:
        with (
            tc.tile_pool(name="const", bufs=1) as cpool,
            tc.tile_pool(name="xw", bufs=1) as xwpool,
            tc.tile_pool(name="qk", bufs=1) as qkpool,
            tc.tile_pool(name="raw", bufs=3) as rawpool,
            tc.tile_pool(name="p", bufs=6) as ppool,
            tc.tile_pool(name="rec", bufs=2) as rpool,
            tc.tile_pool(name="ev", bufs=3) as evpool,
            tc.tile_pool(name="psP", bufs=2, space="PSUM") as psP,
            tc.tile_pool(name="psS", bufs=2, space="PSUM") as psS,
            tc.tile_pool(name="psY", bufs=1, space="PSUM") as psY,
        ):
            # ---- constant / weight loads ----
            # order matters: wq + the first xT column-chunks gate the first
            # matmul groups, so issue them first; xT is loaded in 512-col
            # chunks so projection groups start after ~1MB, not 4MB.
            wq_sb = xwpool.tile([128, 8, HD], BF16, tag="wq")
            wk_sb = xwpool.tile([128, 8, HD], BF16, tag="wk")
            wv_sb = xwpool.tile([128, 8, HD], BF16, tag="wv")
            wo_sb = xwpool.tile([128, 2, D], BF16, tag="wo")
            brow_sb = cpool.tile([1, 2 * HD], BF16, tag="brow")
            ones_sb = cpool.tile([1, 512], BF16, tag="ones")
            bv_sb = cpool.tile([128, HD], F32, tag="bv")
            wq_r = wq.ap().rearrange("(c p) n -> p c n", p=128)
            wk_r = wk.ap().rearrange("(c p) n -> p c n", p=128)
            # pair-0 (c2=0) weight halves first: the first attention quarter
            # needs only these
            nc.sync.dma_start(wq_sb[:, :, 0:128], wq_r[:, :, 0:128])
            nc.sync.dma_start(brow_sb[:], brow.ap())
            nc.vector.memset(ones_sb[:], 1.0)

            xts = []
            for dc in range(8):
                xt = xwpool.tile([128, T], BF16, tag=f"xt{dc}", name=f"xt{dc}")
                xts.append(xt)

            def load_xt_tr(tr):
                for dc in range(8):
                    nc.sync.dma_start(
                        xts[dc][:, tr * 512:(tr + 1) * 512],
                        xT[dc * 128:(dc + 1) * 128, tr * 512:(tr + 1) * 512],
                    )

            cos_sb = cpool.tile([128, T], BF16, tag="cos")
            sin_sb = cpool.tile([128, T], BF16, tag="sin")
            tri2_sb = cpool.tile([128, 2, 128], BF16, tag="tri2")
            kb_sb = cpool.tile([128, NK], F32, tag="kb")
            # everything the first attention quarter needs (q/k tr0 chunks,
            # rope tables, wv, exp consts) comes before the bulk of xT so
            # the lead-in isn't DMA-gated
            load_xt_tr(0)
            nc.sync.dma_start(wk_sb[:, :, 0:128], wk_r[:, :, 0:128])
            nc.sync.dma_start(cos_sb[:, 0:512], cos2[:, 0:512])
            nc.sync.dma_start(sin_sb[:, 0:512], sin2[:, 0:512])
            nc.sync.dma_start(wv_sb[:], wv.ap().rearrange("(c p) n -> p c n", p=128))
            nc.sync.dma_start(
                tri2_sb[:], tri2.ap().rearrange("p (two q) -> p two q", two=2))
            nc.sync.dma_start(kb_sb[:], kb.ap().rearrange("(t p) -> p t", p=128))
            nc.sync.dma_start(bv_sb[:], bv.ap().to_broadcast((128, HD)))
            load_xt_tr(1)
            nc.sync.dma_start(cos_sb[:, 512:T], cos2[:, 512:T])
            nc.sync.dma_start(sin_sb[:, 512:T], sin2[:, 512:T])
            load_xt_tr(2)
            load_xt_tr(3)
            nc.sync.dma_start(wq_sb[:, :, 128:256], wq_r[:, :, 128:256])
            nc.sync.dma_start(wk_sb[:, :, 128:256], wk_r[:, :, 128:256])
            nc.sync.dma_start(wo_sb[:], wo.ap().rearrange("(c p) n -> p c n", p=128))

            # persistent [128, T] tiles: 2 heads each (rows 0:64 / 64:128)
            qT = [qkpool.tile([128, T], BF16, tag=f"qT{c}", name=f"qT{c}") for c in range(2)]
            kT = [qkpool.tile([128, T], BF16, tag=f"kT{c}", name=f"kT{c}") for c in range(2)]
            yT = [qkpool.tile([128, T], BF16, tag=f"yT{c}", name=f"yT{c}") for c in range(2)]

            # ---- Q^T / K^T projection + RoPE, one 512-col chunk ----
            # ti: 0 = q, 1 = k (selects bias row slice)
            def proj_qk_chunk(ti, wsb, c2, fin, tr):
                lo, hi = tr * 512, (tr + 1) * 512
                ps = psP.tile([128, 512], F32, tag="pp")
                for dc in range(8):
                    nc.tensor.matmul(
                        ps[:],
                        wsb[:, dc, c2 * 128:(c2 + 1) * 128],
                        xts[dc][:, lo:hi],
                        start=(dc == 0),
                        stop=False,
                    )
                # bias via rank-1 update: ones row x bias row
                nc.tensor.matmul(
                    ps[:],
                    brow_sb[0:1, ti * HD + c2 * 128: ti * HD + (c2 + 1) * 128],
                    ones_sb[0:1, :],
                    start=False, stop=True,
                )
                raw = rawpool.tile([128, 512], BF16, tag="raw")
                nc.vector.tensor_copy(raw[:], ps[:])
                # RoPE: fin = raw*cos + rot(raw)*sin_signed
                f = fin
                for (do, di) in ((0, 32), (32, 0), (64, 96), (96, 64)):
                    nc.vector.tensor_copy(f[do:do + 32, lo:hi], raw[di:di + 32, :])
                nc.vector.tensor_mul(f[:, lo:hi], f[:, lo:hi], sin_sb[:, lo:hi])
                nc.vector.tensor_mul(raw[:], raw[:], cos_sb[:, lo:hi])
                # final add on GpSimd (idle) to unload DVE
                nc.gpsimd.tensor_add(f[:, lo:hi], f[:, lo:hi], raw[:])

            # same projection for TWO 512-col chunks, dc-major over two psum
            # tiles: each stationary slice serves both chunks back-to-back so
            # _dedup_ldweights deletes every second (identical) weight load.
            def proj_qk_pair(ti, wsb, c2, fin, trp):
                trs = (2 * trp, 2 * trp + 1)
                pss = [psP.tile([128, 512], F32, tag="pp", name=f"pp{t}")
                       for t in range(2)]
                for dc in range(8):
                    for t in range(2):
                        lo = trs[t] * 512
                        nc.tensor.matmul(
                            pss[t][:],
                            wsb[:, dc, c2 * 128:(c2 + 1) * 128],
                            xts[dc][:, lo:lo + 512],
                            start=(dc == 0),
                            stop=False,
                            skip_group_check=True,
                        )
                for t in range(2):
                    nc.tensor.matmul(
                        pss[t][:],
                        brow_sb[0:1, ti * HD + c2 * 128: ti * HD + (c2 + 1) * 128],
                        ones_sb[0:1, :],
                        start=False, stop=True,
                        skip_group_check=True,
                    )
                for t in range(2):
                    lo = trs[t] * 512
                    hi = lo + 512
                    raw = rawpool.tile([128, 512], BF16, tag="raw")
                    nc.vector.tensor_copy(raw[:], pss[t][:])
                    f = fin
                    for (do, di) in ((0, 32), (32, 0), (64, 96), (96, 64)):
                        nc.vector.tensor_copy(f[do:do + 32, lo:hi], raw[di:di + 32, :])
                    nc.vector.tensor_mul(f[:, lo:hi], f[:, lo:hi], sin_sb[:, lo:hi])
                    nc.vector.tensor_mul(raw[:], raw[:], cos_sb[:, lo:hi])
                    nc.gpsimd.tensor_add(f[:, lo:hi], f[:, lo:hi], raw[:])

            # ---- V projection (normal layout, with bias and ones blocks) ----
            vts = [None] * NK

            def proj_v(kt):
                ps = psP.tile([128, HD], F32, tag="pp")
                for dc in range(8):
                    nc.tensor.matmul(
                        ps[:],
                        xts[dc][:, kt * 128:(kt + 1) * 128],
                        wv_sb[:, dc, :],
                        start=(dc == 0),
                        stop=(dc == 7),
                    )
                vt = xwpool.tile([128, HG, 128], BF16, tag=f"v{kt}", name=f"v{kt}")
                nc.vector.tensor_add(
                    vt[:, :, 0:64],
                    ps[:].rearrange("p (h d) -> p h d", h=HG),
                    bv_sb[:].rearrange("p (h d) -> p h d", h=HG),
                )
                nc.vector.memset(vt[:, :, 64:128], 1.0)
                vts[kt] = vt

            # ---- attention for head pair c2, one q-quarter (512 cols) ----
            # scores/probs/Y for both heads live side by side in one
            # [128, 2, 512] tile: [:, 0, :] = head 2*c2, [:, 1, :] = 2*c2+1.
            def attn_quarter(c2, qq):
                qlo = qq * 512
                last = 4 * qq + 3
                y = psY.tile([128, 2, 512], F32, tag="y", name=f"y{c2}_{qq}")
                for kt in range(last + 1):
                    j = kt - 4 * qq
                    c = j * 128 if j >= 0 else 0   # first valid col (diag trim)
                    ksl = slice(kt * 128, (kt + 1) * 128)
                    qsl = slice(qlo + c, qlo + 512)
                    s = psS.tile([128, 2, 512], F32, tag="s")
                    # two row-group-concurrent 64-row score matmuls
                    nc.tensor.matmul(
                        s[:, 0, c:], kT[c2][0:64, ksl], qT[c2][0:64, qsl],
                        start=True, stop=True,
                    )
                    nc.tensor.matmul(
                        s[:, 1, c:], kT[c2][64:128, ksl], qT[c2][64:128, qsl],
                        start=True, stop=True,
                    )
                    p = ppool.tile([128, 2, 512], BF16, tag="p")
                    nc.scalar.activation(
                        p[:, :, c:], s[:, :, c:],
                        mybir.ActivationFunctionType.Exp,
                        bias=kb_sb[:, kt:kt + 1], scale=SCALE,
                    )
                    if j >= 0:
                        # diagonal 128x128 blocks of both heads: tri mask
                        nc.vector.tensor_mul(
                            p[:, :, c:c + 128], p[:, :, c:c + 128], tri2_sb[:]
                        )
                    for h in (0, 1):
                        nc.tensor.matmul(
                            y[:, h, c:],
                            vts[kt][:, 2 * c2 + h, :],
                            p[:, h, c:],
                            start=(kt == 0),
                            stop=(kt == last),
                            skip_group_check=True,
                        )
                # normalize both heads at once: 1/r = exp(-ln r)
                lnr = rpool.tile([64, 2, 512], F32, tag="lnr")
                rec = rpool.tile([64, 2, 512], F32, tag="rec")
                nc.scalar.activation(
                    lnr[:], y[64:128, :, :], mybir.ActivationFunctionType.Ln)
                nc.scalar.activation(
                    rec[:], lnr[:], mybir.ActivationFunctionType.Exp,
                    scale=-1.0)
                nc.vector.tensor_mul(
                    yT[c2][0:64, qlo:qlo + 512], y[0:64, 0, :], rec[:, 0, :])
                nc.vector.tensor_mul(
                    yT[c2][64:128, qlo:qlo + 512], y[0:64, 1, :], rec[:, 1, :])

            def outproj_quarter(qq):
                # partial out for T-tiles of this quarter; host adds bo+reduces
                # c2-outer / dr-inner: each yT stationary slice serves both wo
                # column halves (second load deleted by _dedup_ldweights)
                for tt in range(4 * qq, 4 * qq + 4):
                    pss = [psP.tile([128, 512], F32, tag="pp", name=f"po{t}")
                           for t in range(2)]
                    for c2 in range(2):
                        for dr in range(2):
                            nc.tensor.matmul(
                                pss[dr][:],
                                yT[c2][:, tt * 128:(tt + 1) * 128],
                                wo_sb[:, c2, dr * 512:(dr + 1) * 512],
                                start=(c2 == 0),
                                stop=(c2 == 1),
                                skip_group_check=True,
                            )
                    for dr in range(2):
                        ev = evpool.tile([128, 512], BF16, tag="ev")
                        nc.vector.tensor_copy(ev[:], pss[dr][:])
                        nc.sync.dma_start(
                            out[tt * 128:(tt + 1) * 128, dr * 512:(dr + 1) * 512],
                            ev[:],
                        )

            # ---- emission order == scheduler priority ----
            # quarter qq of pair 0 needs exactly q/k chunks tr<=qq and
            # V tiles kt<=4qq+3, so interleave per quarter: attention
            # starts right after the first 1.5MB of DMA instead of after
            # the whole projection phase.
            # quarters 0/1 use single chunks (tr0 must not wait on tr1's xT
            # DMA in the lead-in); later projections go in reuse-pairs
            for qq in range(2):
                proj_qk_chunk(0, wq_sb, 0, qT[0], qq)
                proj_qk_chunk(1, wk_sb, 0, kT[0], qq)
                for kt in range(4 * qq, 4 * qq + 4):
                    proj_v(kt)
                attn_quarter(0, qq)
            proj_qk_pair(0, wq_sb, 0, qT[0], 1)
            proj_qk_pair(1, wk_sb, 0, kT[0], 1)
            for kt in range(8, 12):
                proj_v(kt)
            attn_quarter(0, 2)
            # V 12-15 before the pair-1 projections: attn(0,3) needs them
            # and the psP ring serves groups in emission order
            for kt in range(12, 16):
                proj_v(kt)
            proj_qk_pair(0, wq_sb, 1, qT[1], 0)
            proj_qk_pair(1, wk_sb, 1, kT[1], 0)
            attn_quarter(0, 3)
            proj_qk_pair(0, wq_sb, 1, qT[1], 1)
            proj_qk_pair(1, wk_sb, 1, kT[1], 1)
            # out-projection emitted one quarter late: its dense matmuls fill
            # the attention exp-wait gaps instead of outranking the next
            # quarter's score matmuls (it shares no psum ring with attention)
            for qq in range(4):
                attn_quarter(1, qq)
                if qq >= 1:
                    outproj_quarter(qq - 1)
            outproj_quarter(3)
    nd = _dedup_ldweights(nc)
    _split_multi_waits(nc)
    assert nd > 0, f"expected ldweights dedup to fire, got {nd}"
    return nc


def _rope_tables():
    inv_freq = 1.0 / (THETA ** (np.arange(0, HS, 2, dtype=np.float64) / HS))  # [32]
    t = np.arange(T, dtype=np.float64)
    fr = t[:, None] * inv_freq[None, :]          # [T, 32]
    emb = np.concatenate([fr, fr], axis=1)       # [T, 64]
    cos = np.cos(emb).T.astype(np.float32)       # [64, T]
    sin = np.sin(emb).T.astype(np.float32)       # [64, T]
    sin_signed = sin.copy()
    sin_signed[0:32] = -sin_signed[0:32]
    cos2 = np.concatenate([cos, cos], axis=0)            # [128, T]
    sin2 = np.concatenate([sin_signed, sin_signed], 0)   # [128, T]
    return cos2.astype(ml_dtypes.bfloat16), sin2.astype(ml_dtypes.bfloat16)


def _in_maps(x, attention_mask, Wq, bqv, Wk, bkv, Wv, bvv, Wo):
    cos2, sin2 = _rope_tables()
    tri = np.triu(np.ones((128, 128), np.float32))
    tri2 = np.concatenate([tri, tri], axis=1).astype(ml_dtypes.bfloat16)
    bf = ml_dtypes.bfloat16
    xTs = [np.ascontiguousarray(x[b].T).astype(bf) for b in range(B)]
    kbs = [
        np.where(attention_mask[b] != 0, 0.0, NEG).astype(np.float32)
        for b in range(B)
    ]
    maps = []
    for core in range(NCORES):
        b, g = core // 4, core % 4
        sl = slice(g * HD, (g + 1) * HD)
        brow = np.concatenate([bqv[sl], bkv[sl]]).reshape(1, 2 * HD)
        maps.append({
            "xT": xTs[b],
            "wq": np.ascontiguousarray(Wq[:, sl]).astype(bf),
            "wk": np.ascontiguousarray(Wk[:, sl]).astype(bf),
            "wv": np.ascontiguousarray(Wv[:, sl]).astype(bf),
            "wo": np.ascontiguousarray(Wo[sl, :]).astype(bf),
            "brow": brow.astype(bf),
            "bv": bvv[sl].astype(np.float32).reshape(1, HD),
            "cos2": cos2,
            "sin2": sin2,
            "tri2": tri2,
            "kb": kbs[b],
        })
    return maps


def _run(inputs, trace=False):
    global _NC
    if _NC is None:
        _NC = build_nc()
    maps = _in_maps(
        np.asarray(inputs["x"]), np.asarray(inputs["attention_mask"]),
        np.asarray(inputs["Wq"]), np.asarray(inputs["bq"]),
        np.asarray(inputs["Wk"]), np.asarray(inputs["bk"]),
        np.asarray(inputs["Wv"]), np.asarray(inputs["bv"]),
        np.asarray(inputs["Wo"]),
    )
    res = run_bass_kernel_spmd(_NC, maps, core_ids=list(range(NCORES)), trace=trace)
    bo = np.asarray(inputs["bo"], np.float32)
    outs = []
    for b in range(B):
        acc = np.zeros((T, D), np.float32)
        for g in range(4):
            acc += np.asarray(res.results[b * 4 + g]["out"], np.float32)
        outs.append(acc + bo[None, :])
    return np.stack(outs, axis=0), res


def kernel(**inputs):
    out, _ = _run(inputs, trace=False)
    return out


# revision 29
# speedup vs baseline: 1.1541x; 1.0067x over previous
"""Distributed Trainium2 Bass kernel for a 16-head causal RoPE attention layer.

Problem: B=2, T=2048, D=1024, H=16, HS=64 (fp32 reference).

Sharding (8 cores): core = b*4 + g, b in {0,1} (batch), g in {0..3} (group of
4 heads).  Each core computes Q/K/V projections for its 256 head-dims, runs
causal flash-style attention for its 4 heads, and applies its 256-row slice
of Wo, producing a partial [T, D] output.  The host sums the 4 partials per
batch and adds bo.  No on-device collectives.

Pipeline (v2): the softmax exp on ScalarE is the throughput floor
(~55us/core of pure data at 1 elem/lane/cycle), so the whole kernel is
arranged to keep ScalarE maximally busy on exp and everything else
overlapped under it:
  - scores for a HEAD PAIR share one [128, 2x512] PSUM tile (head A cols
    0:512, head B 512:1024) written by two row-group-concurrent 64-row
    matmuls (tile_position (0,0)/(64,0) auto-derived), so each exp op
    covers 1024 cols -> half the ACT per-op pipeline overhead.
  - attention is q-quarter major (512 q cols per step) so Y for both
    heads fits one [128, 2x512] PSUM tile; flash accumulation over k
    tiles; denominators via 64 ones-columns in the V stationary.
  - q/k biases are folded into the projection matmuls as a 9th
    contraction row (ones moving row), evictions are pure DVE casts.
  - RoPE runs in 512-col chunks (cast -> 4 DVE partition-shift copies ->
    2 DVE muls -> GpSimd add) so attention starts early.
  - normalize: 1/r = exp(-ln r) on ScalarE over both heads at once.
  - diagonal 128x128 blocks: exp trimmed to the valid cols, triangular
    mask applied multiplicatively on DVE.
PSUM: scores 2 bufs x 2 banks + Y 1 buf x 2 banks + proj/outproj ring
2 bufs x 1 bank = 8 banks exactly.
"""

import numpy as np
import ml_dtypes

import concourse.bass as bass
import concourse.mybir as mybir
import concourse.tile as tile
from concourse.bass_utils import run_bass_kernel_spmd

BF16 = mybir.dt.bfloat16
F32 = mybir.dt.float32

B, T, D = 2, 2048, 1024
H, HS = 16, 64
THETA = 10000.0
NCORES = 8
HG = 4            # heads per core
HD = HG * HS      # head dims per core = 256
SCALE = 1.0 / 8.0  # 1/sqrt(HS)
NEG = -1.0e5       # additive mask for padded keys (exp underflows to 0)

_NC = None


_SELF_SEM = {
    "EngineType.Activation": "Activation_",
    "EngineType.DVE": "DVE_",
    "EngineType.PE": "PE_",
    "EngineType.Pool": "Pool_",
}


def _split_multi_waits(nc):
    """walrus codegen accepts at most ONE semaphore wait per engine
    instruction (the 64B ISA structs have a single EVENTS slot); Tile's
    scheduler freely emits several.  Hoist all but the last wait of each
    instruction onto inserted same-engine EventSemaphore (poll_sem) ops,
    which preserves semantics exactly (engines execute sequentially).

    Additionally drop ge-waits on the instruction's OWN engine semaphore
    for compute engines: those guard WAW/WAR against earlier instructions
    of the same in-order engine, which program order already guarantees
    (each op's writes drain before the next op's visible effects).  Tile
    emits one before nearly every exp in the attention loop; at ~100ns of
    sequencer dispatch each they are pure overhead."""
    def _names(args):
        out = set()
        for a in args:
            for attr in ("memref", "name"):
                v = getattr(a, attr, None)
                if isinstance(v, str):
                    out.add(v.removesuffix("_set"))
            t = getattr(a, "tensor", None)
            if t is not None and isinstance(getattr(t, "name", None), str):
                out.add(t.name)
        return out

    # per-engine written/read tensor sets: an op READING an own-engine-
    # written tensor (RAW) or WRITING an own-engine-read tensor (WAR) has a
    # genuine same-engine hazard through the deep pipeline, so its self-wait
    # must survive; pure WAW through the in-order write port is safe.
    eng_written = {}
    eng_read = {}
    _COMPUTE = {"InstActivation", "InstTensorTensor", "InstTensorCopy",
                "InstMatmult", "InstLdweights", "InstMemset",
                "InstTensorScalarPtr", "InstTensorReduce"}
    for f in nc.m.functions:
        for blk in f.blocks:
            for inst in blk.instructions:
                if type(inst).__name__ in _COMPUTE:
                    e = str(inst.engine)
                    eng_written.setdefault(e, set()).update(_names(inst.outs))
                    eng_read.setdefault(e, set()).update(_names(inst.ins))

    n = 0
    for f in nc.m.functions:
        for blk in f.blocks:
            il = blk.instructions
            i = 0
            while i < len(il):
                inst = il[i]
                si = inst.sync_info
                if si is None or not si.on_wait:
                    i += 1
                    continue
                waits = list(si.on_wait)
                eng = str(inst.engine)
                selfpfx = _SELF_SEM.get(eng)
                if (selfpfx is not None
                        and type(inst).__name__ == "InstActivation"
                        and not (_names(inst.ins) & eng_written.get(eng, set()))
                        and not (_names(inst.outs) & eng_read.get(eng, set()))):
                    kept = [w for w in waits
                            if not (w.wait_mode == "sem-ge-imm"
                                    and w.ant_name.startswith(selfpfx))]
                    if len(kept) != len(waits):
                        waits = kept
                        inst.sync_info = mybir.SyncInfo(
                            on_wait=waits, on_update=list(si.on_update))
                if len(waits) > 1:
                    for w in waits[:-1]:
                        es = mybir.InstEventSemaphore(name=f"I-wsplit-{n}")
                        n += 1
                        es.engine = inst.engine
                        es.sync_info = mybir.SyncInfo(on_wait=[w], on_update=[])
                        nc.register_instruction(es)
                        il.insert(i, es)
                        i += 1
                    inst.sync_info = mybir.SyncInfo(
                        on_wait=[waits[-1]], on_update=list(si.on_update))
                i += 1
    return n


def _dedup_ldweights(nc):
    """bass emits one InstLdweights per InstMatmult.  When a later
    InstLdweights loads the IDENTICAL weights AP that is already resident
    in the PE array (no other InstLdweights in between), the reload is
    redundant: MATMUL does not self-load for 16-bit dtypes.  Delete it,
    folding its waits into the following matmult (whose multi-waits are
    then legalized by _split_multi_waits)."""
    def fp(inst):
        a = inst.ins[0]
        return (a.memref, a.offset, str(a.ap), str(a.dtype))

    n = 0
    for f in nc.m.functions:
        for blk in f.blocks:
            il = blk.instructions
            last = None
            i = 0
            while i < len(il):
                inst = il[i]
                tn = type(inst).__name__
                if tn == "InstLdweights":
                    cur = fp(inst)
                    si = inst.sync_info
                    if cur == last and (si is None or not si.on_update):
                        waits = list(si.on_wait) if si is not None else []
                        if waits:
                            j = i + 1
                            while (j < len(il)
                                   and type(il[j]).__name__ != "InstMatmult"):
                                j += 1
                            if j == len(il):
                                i += 1
                                continue
                            mm = il[j]
                            msi = mm.sync_info
                            mw = list(msi.on_wait) if msi is not None else []
                            mu = list(msi.on_update) if msi is not None else []
                            mm.sync_info = mybir.SyncInfo(
                                on_wait=waits + mw, on_update=mu)
                        del il[i]
                        n += 1
                        continue
                    last = cur
                i += 1
    return n


def build_nc():
    nc = bass.Bass()

    xT = nc.declare_dram_parameter("xT", [D, T], BF16, isOutput=False)
    wq = nc.declare_dram_parameter("wq", [D, HD], BF16, isOutput=False)
    wk = nc.declare_dram_parameter("wk", [D, HD], BF16, isOutput=False)
    wv = nc.declare_dram_parameter("wv", [D, HD], BF16, isOutput=False)
    wo = nc.declare_dram_parameter("wo", [HD, D], BF16, isOutput=False)
    # [bq(256) | bk(256)] as a single stationary bias row
    brow = nc.declare_dram_parameter("brow", [1, 2 * HD], BF16, isOutput=False)
    bv = nc.declare_dram_parameter("bv", [1, HD], F32, isOutput=False)
    cos2 = nc.declare_dram_parameter("cos2", [128, T], BF16, isOutput=False)
    sin2 = nc.declare_dram_parameter("sin2", [128, T], BF16, isOutput=False)
    tri2 = nc.declare_dram_parameter("tri2", [128, 256], BF16, isOutput=False)
    kb = nc.declare_dram_parameter("kb", [T], F32, isOutput=False)
    out = nc.declare_dram_parameter("out", [T, D], BF16, isOutput=True)

    NT = T // 512   # 4 T-ranges for projections
    NK = T // 128   # 16 key tiles

    with tile.TileContext(nc) as tc:
        with (
            tc.tile_pool(name="const", bufs=1) as cpool,
            tc.tile_pool(name="xw", bufs=1) as xwpool,
            tc.tile_pool(name="qk", bufs=1) as qkpool,
            tc.tile_pool(name="raw", bufs=3) as rawpool,
            tc.tile_pool(name="p", bufs=6) as ppool,
            tc.tile_pool(name="rec", bufs=2) as rpool,
            tc.tile_pool(name="ev", bufs=3) as evpool,
            tc.tile_pool(name="psP", bufs=2, space="PSUM") as psP,
            tc.tile_pool(name="psS", bufs=2, space="PSUM") as psS,
            tc.tile_pool(name="psY", bufs=1, space="PSUM") as psY,
        ):
            # ---- constant / weight loads ----
            # order matters: wq + the first xT column-chunks gate the first
            # matmul groups, so issue them first; xT is loaded in 512-col
            # chunks so projection groups start after ~1MB, not 4MB.
            wq_sb = xwpool.tile([128, 8, HD], BF16, tag="wq")
            wk_sb = xwpool.tile([128, 8, HD], BF16, tag="wk")
            wv_sb = xwpool.tile([128, 8, HD], BF16, tag="wv")
            wo_sb = xwpool.tile([128, 2, D], BF16, tag="wo")
            brow_sb = cpool.tile([1, 2 * HD], BF16, tag="brow")
            ones_sb = cpool.tile([1, 512], BF16, tag="ones")
            bv_sb = cpool.tile([128, HD], F32, tag="bv")
            wq_r = wq.ap().rearrange("(c p) n -> p c n", p=128)
            wk_r = wk.ap().rearrange("(c p) n -> p c n", p=128)
            # pair-0 (c2=0) weight halves first: the first attention quarter
            # needs only these
            nc.sync.dma_start(wq_sb[:, :, 0:128], wq_r[:, :, 0:128])
            nc.sync.dma_start(brow_sb[:], brow.ap())
            nc.vector.memset(ones_sb[:], 1.0)

            xts = []
            for dc in range(8):
                xt = xwpool.tile([128, T], BF16, tag=f"xt{dc}", name=f"xt{dc}")
                xts.append(xt)

            def load_xt_tr(tr):
                for dc in range(8):
                    nc.sync.dma_start(
                        xts[dc][:, tr * 512:(tr + 1) * 512],
                        xT[dc * 128:(dc + 1) * 128, tr * 512:(tr + 1) * 512],
                    )

            cos_sb = cpool.tile([128, T], BF16, tag="cos")
            sin_sb = cpool.tile([128, T], BF16, tag="sin")
            tri2_sb = cpool.tile([128, 2, 128], BF16, tag="tri2")
            kb_sb = cpool.tile([128, NK], F32, tag="kb")
            # everything the first attention quarter needs (q/k tr0 chunks,
            # rope tables, wv, exp consts) comes before the bulk of xT so
            # the lead-in isn't DMA-gated
            load_xt_tr(0)
            nc.sync.dma_start(wk_sb[:, :, 0:128], wk_r[:, :, 0:128])
            nc.sync.dma_start(cos_sb[:, 0:512], cos2[:, 0:512])
            nc.sync.dma_start(sin_sb[:, 0:512], sin2[:, 0:512])
            nc.sync.dma_start(wv_sb[:], wv.ap().rearrange("(c p) n -> p c n", p=128))
            nc.sync.dma_start(
                tri2_sb[:], tri2.ap().rearrange("p (two q) -> p two q", two=2))
            nc.sync.dma_start(kb_sb[:], kb.ap().rearrange("(t p) -> p t", p=128))
            nc.sync.dma_start(bv_sb[:], bv.ap().to_broadcast((128, HD)))
            load_xt_tr(1)
            nc.sync.dma_start(cos_sb[:, 512:T], cos2[:, 512:T])
            nc.sync.dma_start(sin_sb[:, 512:T], sin2[:, 512:T])
            load_xt_tr(2)
            load_xt_tr(3)
            nc.sync.dma_start(wq_sb[:, :, 128:256], wq_r[:, :, 128:256])
            nc.sync.dma_start(wk_sb[:, :, 128:256], wk_r[:, :, 128:256])
            nc.sync.dma_start(wo_sb[:], wo.ap().rearrange("(c p) n -> p c n", p=128))

            # persistent [128, T] tiles: 2 heads each (rows 0:64 / 64:128)
            qT = [qkpool.tile([128, T], BF16, tag=f"qT{c}", name=f"qT{c}") for c in range(2)]
            kT = [qkpool.tile([128, T], BF16, tag=f"kT{c}", name=f"kT{c}") for c in range(2)]
            yT = [qkpool.tile([128, T], BF16, tag=f"yT{c}", name=f"yT{c}") for c in range(2)]

            # ---- Q^T / K^T projection + RoPE, one 512-col chunk ----
            # ti: 0 = q, 1 = k (selects bias row slice)
            def proj_qk_chunk(ti, wsb, c2, fin, tr):
                lo, hi = tr * 512, (tr + 1) * 512
                ps = psP.tile([128, 512], F32, tag="pp")
                for dc in range(8):
                    nc.tensor.matmul(
                        ps[:],
                        wsb[:, dc, c2 * 128:(c2 + 1) * 128],
                        xts[dc][:, lo:hi],
                        start=(dc == 0),
                        stop=False,
                    )
                # bias via rank-1 update: ones row x bias row
                nc.tensor.matmul(
                    ps[:],
                    brow_sb[0:1, ti * HD + c2 * 128: ti * HD + (c2 + 1) * 128],
                    ones_sb[0:1, :],
                    start=False, stop=True,
                )
                raw = rawpool.tile([128, 512], BF16, tag="raw")
                nc.vector.tensor_copy(raw[:], ps[:])
                # RoPE: fin = raw*cos + rot(raw)*sin_signed
                f = fin
                for (do, di) in ((0, 32), (32, 0), (64, 96), (96, 64)):
                    nc.vector.tensor_copy(f[do:do + 32, lo:hi], raw[di:di + 32, :])
                nc.vector.tensor_mul(f[:, lo:hi], f[:, lo:hi], sin_sb[:, lo:hi])
                nc.vector.tensor_mul(raw[:], raw[:], cos_sb[:, lo:hi])
                # final add on GpSimd (idle) to unload DVE
                nc.gpsimd.tensor_add(f[:, lo:hi], f[:, lo:hi], raw[:])

            # same projection for TWO 512-col chunks, dc-major over two psum
            # tiles: each stationary slice serves both chunks back-to-back so
            # _dedup_ldweights deletes every second (identical) weight load.
            def proj_qk_pair(ti, wsb, c2, fin, trp):
                trs = (2 * trp, 2 * trp + 1)
                pss = [psP.tile([128, 512], F32, tag="pp", name=f"pp{t}")
                       for t in range(2)]
                for dc in range(8):
                    for t in range(2):
                        lo = trs[t] * 512
                        nc.tensor.matmul(
                            pss[t][:],
                            wsb[:, dc, c2 * 128:(c2 + 1) * 128],
                            xts[dc][:, lo:lo + 512],
                            start=(dc == 0),
                            stop=False,
                            skip_group_check=True,
                        )
                for t in range(2):
                    nc.tensor.matmul(
                        pss[t][:],
                        brow_sb[0:1, ti * HD + c2 * 128: ti * HD + (c2 + 1) * 128],
                        ones_sb[0:1, :],
                        start=False, stop=True,
                        skip_group_check=True,
                    )
                for t in range(2):
                    lo = trs[t] * 512
                    hi = lo + 512
                    raw = rawpool.tile([128, 512], BF16, tag="raw")
                    nc.vector.tensor_copy(raw[:], pss[t][:])
                    f = fin
                    for (do, di) in ((0, 32), (32, 0), (64, 96), (96, 64)):
                        nc.vector.tensor_copy(f[do:do + 32, lo:hi], raw[di:di + 32, :])
                    nc.vector.tensor_mul(f[:, lo:hi], f[:, lo:hi], sin_sb[:, lo:hi])
                    nc.vector.tensor_mul(raw[:], raw[:], cos_sb[:, lo:hi])
                    nc.gpsimd.tensor_add(f[:, lo:hi], f[:, lo:hi], raw[:])

            # ---- V projection (normal layout, with bias and ones blocks) ----
            vts = [None] * NK

            def proj_v(kt):
                ps = psP.tile([128, HD], F32, tag="pp")
                for dc in range(8):
                    nc.tensor.matmul(
                        ps[:],
                        xts[dc][:, kt * 128:(kt + 1) * 128],
                        wv_sb[:, dc, :],
                        start=(dc == 0),
                        stop=(dc == 7),
                    )
                vt = xwpool.tile([128, HG, 128], BF16, tag=f"v{kt}", name=f"v{kt}")
                nc.vector.tensor_add(
                    vt[:, :, 0:64],
                    ps[:].rearrange("p (h d) -> p h d", h=HG),
                    bv_sb[:].rearrange("p (h d) -> p h d", h=HG),
                )
                nc.vector.memset(vt[:, :, 64:128], 1.0)
                vts[kt] = vt

            # ---- attention for head pair c2, one q-quarter (512 cols) ----
            # scores/probs/Y for both heads live side by side in one
            # [128, 2, 512] tile: [:, 0, :] = head 2*c2, [:, 1, :] = 2*c2+1.
            def attn_quarter(c2, qq):
                qlo = qq * 512
                last = 4 * qq + 3
                y = psY.tile([128, 2, 512], F32, tag="y", name=f"y{c2}_{qq}")
                for kt in range(last + 1):
                    j = kt - 4 * qq
                    c = j * 128 if j >= 0 else 0   # first valid col (diag trim)
                    ksl = slice(kt * 128, (kt + 1) * 128)
                    qsl = slice(qlo + c, qlo + 512)
                    s = psS.tile([128, 2, 512], F32, tag="s")
                    # two row-group-concurrent 64-row score matmuls
                    nc.tensor.matmul(
                        s[:, 0, c:], kT[c2][0:64, ksl], qT[c2][0:64, qsl],
                        start=True, stop=True,
                    )
                    nc.tensor.matmul(
                        s[:, 1, c:], kT[c2][64:128, ksl], qT[c2][64:128, qsl],
                        start=True, stop=True,
                    )
                    p = ppool.tile([128, 2, 512], BF16, tag="p")
                    nc.scalar.activation(
                        p[:, :, c:], s[:, :, c:],
                        mybir.ActivationFunctionType.Exp,
                        bias=kb_sb[:, kt:kt + 1], scale=SCALE,
                    )
                    if j >= 0:
                        # diagonal 128x128 blocks of both heads: tri mask
                        nc.vector.tensor_mul(
                            p[:, :, c:c + 128], p[:, :, c:c + 128], tri2_sb[:]
                        )
                    for h in (0, 1):
                        nc.tensor.matmul(
                            y[:, h, c:],
                            vts[kt][:, 2 * c2 + h, :],
                            p[:, h, c:],
                            start=(kt == 0),
                            stop=(kt == last),
                            skip_group_check=True,
                        )
                # normalize both heads at once: 1/r = exp(-ln r)
                lnr = rpool.tile([64, 2, 512], F32, tag="lnr")
                rec = rpool.tile([64, 2, 512], F32, tag="rec")
                nc.scalar.activation(
                    lnr[:], y[64:128, :, :], mybir.ActivationFunctionType.Ln)
                nc.scalar.activation(
                    rec[:], lnr[:], mybir.ActivationFunctionType.Exp,
                    scale=-1.0)
                nc.vector.tensor_mul(
                    yT[c2][0:64, qlo:qlo + 512], y[0:64, 0, :], rec[:, 0, :])
                nc.vector.tensor_mul(
                    yT[c2][64:128, qlo:qlo + 512], y[0:64, 1, :], rec[:, 1, :])

            def outproj_quarter(qq):
                # partial out for T-tiles of this quarter; host adds bo+reduces
                # c2-outer / dr-inner: each yT stationary slice serves both wo
                # column halves (second load deleted by _dedup_ldweights)
                for tt in range(4 * qq, 4 * qq + 4):
                    pss = [psP.tile([128, 512], F32, tag="pp", name=f"po{t}")
                           for t in range(2)]
                    for c2 in range(2):
                        for dr in range(2):
                            nc.tensor.matmul(
                                pss[dr][:],
                                yT[c2][:, tt * 128:(tt + 1) * 128],
                                wo_sb[:, c2, dr * 512:(dr + 1) * 512],
                                start=(c2 == 0),
                                stop=(c2 == 1),
                                skip_group_check=True,
                            )
                    for dr in range(2):
                        ev = evpool.tile([128, 512], BF16, tag="ev")
                        nc.vector.tensor_copy(ev[:], pss[dr][:])
                        nc.sync.dma_start(
                            out[tt * 128:(tt + 1) * 128, dr * 512:(dr + 1) * 512],
                            ev[:],
                        )

            # ---- emission order == scheduler priority ----
            # quarter qq of pair 0 needs exactly q/k chunks tr<=qq and
            # V tiles kt<=4qq+3, so interleave per quarter: attention
            # starts right after the first 1.5MB of DMA instead of after
            # the whole projection phase.
            # quarters 0/1 use single chunks (tr0 must not wait on tr1's xT
            # DMA in the lead-in); later projections go in reuse-pairs
            for qq in range(2):
                proj_qk_chunk(0, wq_sb, 0, qT[0], qq)
                proj_qk_chunk(1, wk_sb, 0, kT[0], qq)
                for kt in range(4 * qq, 4 * qq + 4):
                    proj_v(kt)
                attn_quarter(0, qq)
            proj_qk_pair(0, wq_sb, 0, qT[0], 1)
            proj_qk_pair(1, wk_sb, 0, kT[0], 1)
            for kt in range(8, 12):
                proj_v(kt)
            attn_quarter(0, 2)
            # V 12-15 before the pair-1 projections: attn(0,3) needs them
            # and the psP ring serves groups in emission order
            for kt in range(12, 16):
                proj_v(kt)
            attn_quarter(0, 3)
            # pair-1 tr0/tr1 projections run during attn(0,3) + transition;
            # tr2/tr3 move into the pair-1 window (which has PE slack) so
            # they stop crowding the pair-0 crunch region.  outproj lands
            # one quarter late so its matmuls fill exp-wait gaps instead of
            # outranking the next quarter's score matmuls.
            proj_qk_pair(0, wq_sb, 1, qT[1], 0)
            proj_qk_pair(1, wk_sb, 1, kT[1], 0)
            attn_quarter(1, 0)
            proj_qk_pair(0, wq_sb, 1, qT[1], 1)
            attn_quarter(1, 1)
            proj_qk_pair(1, wk_sb, 1, kT[1], 1)
            outproj_quarter(0)
            attn_quarter(1, 2)
            outproj_quarter(1)
            attn_quarter(1, 3)
            outproj_quarter(2)
            outproj_quarter(3)
    nd = _dedup_ldweights(nc)
    _split_multi_waits(nc)
    assert nd > 0, f"expected ldweights dedup to fire, got {nd}"
    return nc


def _rope_tables():
    inv_freq = 1.0 / (THETA ** (np.arange(0, HS, 2, dtype=np.float64) / HS))  # [32]
    t = np.arange(T, dtype=np.float64)
    fr = t[:, None] * inv_freq[None, :]          # [T, 32]
    emb = np.concatenate([fr, fr], axis=1)       # [T, 64]
    cos = np.cos(emb).T.astype(np.float32)       # [64, T]
    sin = np.sin(emb).T.astype(np.float32)       # [64, T]
    sin_signed = sin.copy()
    sin_signed[0:32] = -sin_signed[0:32]
    cos2 = np.concatenate([cos, cos], axis=0)            # [128, T]
    sin2 = np.concatenate([sin_signed, sin_signed], 0)   # [128, T]
    return cos2.astype(ml_dtypes.bfloat16), sin2.astype(ml_dtypes.bfloat16)


def _in_maps(x, attention_mask, Wq, bqv, Wk, bkv, Wv, bvv, Wo):
    cos2, sin2 = _rope_tables()
    tri = np.triu(np.ones((128, 128), np.float32))
    tri2 = np.concatenate([tri, tri], axis=1).astype(ml_dtypes.bfloat16)
    bf = ml_dtypes.bfloat16
    xTs = [np.ascontiguousarray(x[b].T).astype(bf) for b in range(B)]
    kbs = [
        np.where(attention_mask[b] != 0, 0.0, NEG).astype(np.float32)
        for b in range(B)
    ]
    maps = []
    for core in range(NCORES):
        b, g = core // 4, core % 4
        sl = slice(g * HD, (g + 1) * HD)
        brow = np.concatenate([bqv[sl], bkv[sl]]).reshape(1, 2 * HD)
        maps.append({
            "xT": xTs[b],
            "wq": np.ascontiguousarray(Wq[:, sl]).astype(bf),
            "wk": np.ascontiguousarray(Wk[:, sl]).astype(bf),
            "wv": np.ascontiguousarray(Wv[:, sl]).astype(bf),
            "wo": np.ascontiguousarray(Wo[sl, :]).astype(bf),
            "brow": brow.astype(bf),
            "bv": bvv[sl].astype(np.float32).reshape(1, HD),
            "cos2": cos2,
            "sin2": sin2,
            "tri2": tri2,
            "kb": kbs[b],
        })
    return maps


def _run(inputs, trace=False):
    global _NC
    if _NC is None:
        _NC = build_nc()
    maps = _in_maps(
        np.asarray(inputs["x"]), np.asarray(inputs["attention_mask"]),
        np.asarray(inputs["Wq"]), np.asarray(inputs["bq"]),
        np.asarray(inputs["Wk"]), np.asarray(inputs["bk"]),
        np.asarray(inputs["Wv"]), np.asarray(inputs["bv"]),
        np.asarray(inputs["Wo"]),
    )
    res = run_bass_kernel_spmd(_NC, maps, core_ids=list(range(NCORES)), trace=trace)
    bo = np.asarray(inputs["bo"], np.float32)
    outs = []
    for b in range(B):
        acc = np.zeros((T, D), np.float32)
        for g in range(4):
            acc += np.asarray(res.results[b * 4 + g]["out"], np.float32)
        outs.append(acc + bo[None, :])
    return np.stack(outs, axis=0), res


def kernel(**inputs):
    out, _ = _run(inputs, trace=False)
    return out


# revision 31
# speedup vs baseline: 1.1637x; 1.0083x over previous
"""Distributed Trainium2 Bass kernel for a 16-head causal RoPE attention layer.

Problem: B=2, T=2048, D=1024, H=16, HS=64 (fp32 reference).

Sharding (8 cores): core = b*4 + g, b in {0,1} (batch), g in {0..3} (group of
4 heads).  Each core computes Q/K/V projections for its 256 head-dims, runs
causal flash-style attention for its 4 heads, and applies its 256-row slice
of Wo, producing a partial [T, D] output.  The host sums the 4 partials per
batch and adds bo.  No on-device collectives.

Pipeline (v2): the softmax exp on ScalarE is the throughput floor
(~55us/core of pure data at 1 elem/lane/cycle), so the whole kernel is
arranged to keep ScalarE maximally busy on exp and everything else
overlapped under it:
  - scores for a HEAD PAIR share one [128, 2x512] PSUM tile (head A cols
    0:512, head B 512:1024) written by two row-group-concurrent 64-row
    matmuls (tile_position (0,0)/(64,0) auto-derived), so each exp op
    covers 1024 cols -> half the ACT per-op pipeline overhead.
  - attention is q-quarter major (512 q cols per step) so Y for both
    heads fits one [128, 2x512] PSUM tile; flash accumulation over k
    tiles; denominators via 64 ones-columns in the V stationary.
  - q/k biases are folded into the projection matmuls as a 9th
    contraction row (ones moving row), evictions are pure DVE casts.
  - RoPE runs in 512-col chunks (cast -> 4 DVE partition-shift copies ->
    2 DVE muls -> GpSimd add) so attention starts early.
  - normalize: 1/r = exp(-ln r) on ScalarE over both heads at once.
  - diagonal 128x128 blocks: exp trimmed to the valid cols, triangular
    mask applied multiplicatively on DVE.
PSUM: scores 2 bufs x 2 banks + Y 1 buf x 2 banks + proj/outproj ring
2 bufs x 1 bank = 8 banks exactly.
"""

import numpy as np
import ml_dtypes

import concourse.bass as bass
import concourse.mybir as mybir
import concourse.tile as tile
from concourse.bass_utils import run_bass_kernel_spmd

BF16 = mybir.dt.bfloat16
F32 = mybir.dt.float32

B, T, D = 2, 2048, 1024
H, HS = 16, 64
THETA = 10000.0
NCORES = 8
HG = 4            # heads per core
HD = HG * HS      # head dims per core = 256
SCALE = 1.0 / 8.0  # 1/sqrt(HS)
NEG = -1.0e5       # additive mask for padded keys (exp underflows to 0)

_NC = None


_SELF_SEM = {
    "EngineType.Activation": "Activation_",
    "EngineType.DVE": "DVE_",
    "EngineType.PE": "PE_",
    "EngineType.Pool": "Pool_",
}


def _split_multi_waits(nc):
    """walrus codegen accepts at most ONE semaphore wait per engine
    instruction (the 64B ISA structs have a single EVENTS slot); Tile's
    scheduler freely emits several.  Hoist all but the last wait of each
    instruction onto inserted same-engine EventSemaphore (poll_sem) ops,
    which preserves semantics exactly (engines execute sequentially).

    Additionally drop ge-waits on the instruction's OWN engine semaphore
    for compute engines: those guard WAW/WAR against earlier instructions
    of the same in-order engine, which program order already guarantees
    (each op's writes drain before the next op's visible effects).  Tile
    emits one before nearly every exp in the attention loop; at ~100ns of
    sequencer dispatch each they are pure overhead."""
    def _names(args):
        out = set()
        for a in args:
            for attr in ("memref", "name"):
                v = getattr(a, attr, None)
                if isinstance(v, str):
                    out.add(v.removesuffix("_set"))
            t = getattr(a, "tensor", None)
            if t is not None and isinstance(getattr(t, "name", None), str):
                out.add(t.name)
        return out

    # per-engine written/read tensor sets: an op READING an own-engine-
    # written tensor (RAW) or WRITING an own-engine-read tensor (WAR) has a
    # genuine same-engine hazard through the deep pipeline, so its self-wait
    # must survive; pure WAW through the in-order write port is safe.
    eng_written = {}
    eng_read = {}
    _COMPUTE = {"InstActivation", "InstTensorTensor", "InstTensorCopy",
                "InstMatmult", "InstLdweights", "InstMemset",
                "InstTensorScalarPtr", "InstTensorReduce"}
    for f in nc.m.functions:
        for blk in f.blocks:
            for inst in blk.instructions:
                if type(inst).__name__ in _COMPUTE:
                    e = str(inst.engine)
                    eng_written.setdefault(e, set()).update(_names(inst.outs))
                    eng_read.setdefault(e, set()).update(_names(inst.ins))

    n = 0
    for f in nc.m.functions:
        for blk in f.blocks:
            il = blk.instructions
            i = 0
            while i < len(il):
                inst = il[i]
                si = inst.sync_info
                if si is None or not si.on_wait:
                    i += 1
                    continue
                waits = list(si.on_wait)
                eng = str(inst.engine)
                selfpfx = _SELF_SEM.get(eng)
                if (selfpfx is not None
                        and type(inst).__name__ == "InstActivation"
                        and not (_names(inst.ins) & eng_written.get(eng, set()))
                        and not (_names(inst.outs) & eng_read.get(eng, set()))):
                    kept = [w for w in waits
                            if not (w.wait_mode == "sem-ge-imm"
                                    and w.ant_name.startswith(selfpfx))]
                    if len(kept) != len(waits):
                        waits = kept
                        inst.sync_info = mybir.SyncInfo(
                            on_wait=waits, on_update=list(si.on_update))
                if len(waits) > 1:
                    for w in waits[:-1]:
                        es = mybir.InstEventSemaphore(name=f"I-wsplit-{n}")
                        n += 1
                        es.engine = inst.engine
                        es.sync_info = mybir.SyncInfo(on_wait=[w], on_update=[])
                        nc.register_instruction(es)
                        il.insert(i, es)
                        i += 1
                    inst.sync_info = mybir.SyncInfo(
                        on_wait=[waits[-1]], on_update=list(si.on_update))
                i += 1
    return n


def _dedup_ldweights(nc):
    """bass emits one InstLdweights per InstMatmult.  When a later
    InstLdweights loads the IDENTICAL weights AP that is already resident
    in the PE array (no other InstLdweights in between), the reload is
    redundant: MATMUL does not self-load for 16-bit dtypes.  Delete it,
    folding its waits into the following matmult (whose multi-waits are
    then legalized by _split_multi_waits)."""
    def fp(inst):
        a = inst.ins[0]
        return (a.memref, a.offset, str(a.ap), str(a.dtype))

    n = 0
    for f in nc.m.functions:
        for blk in f.blocks:
            il = blk.instructions
            last = None
            i = 0
            while i < len(il):
                inst = il[i]
                tn = type(inst).__name__
                if tn == "InstLdweights":
                    cur = fp(inst)
                    si = inst.sync_info
                    if cur == last and (si is None or not si.on_update):
                        waits = list(si.on_wait) if si is not None else []
                        if waits:
                            j = i + 1
                            while (j < len(il)
                                   and type(il[j]).__name__ != "InstMatmult"):
                                j += 1
                            if j == len(il):
                                i += 1
                                continue
                            mm = il[j]
                            msi = mm.sync_info
                            mw = list(msi.on_wait) if msi is not None else []
                            mu = list(msi.on_update) if msi is not None else []
                            mm.sync_info = mybir.SyncInfo(
                                on_wait=waits + mw, on_update=mu)
                        del il[i]
                        n += 1
                        continue
                    last = cur
                i += 1
    return n


def build_nc():
    nc = bass.Bass()

    xT = nc.declare_dram_parameter("xT", [D, T], BF16, isOutput=False)
    wq = nc.declare_dram_parameter("wq", [D, HD], BF16, isOutput=False)
    wk = nc.declare_dram_parameter("wk", [D, HD], BF16, isOutput=False)
    wv = nc.declare_dram_parameter("wv", [D, HD], BF16, isOutput=False)
    wo = nc.declare_dram_parameter("wo", [HD, D], BF16, isOutput=False)
    # [bq(256) | bk(256)] as a single stationary bias row
    brow = nc.declare_dram_parameter("brow", [1, 2 * HD], BF16, isOutput=False)
    bv = nc.declare_dram_parameter("bv", [1, HD], F32, isOutput=False)
    cos2 = nc.declare_dram_parameter("cos2", [128, T], BF16, isOutput=False)
    sin2 = nc.declare_dram_parameter("sin2", [128, T], BF16, isOutput=False)
    tri2 = nc.declare_dram_parameter("tri2", [128, 256], BF16, isOutput=False)
    kb = nc.declare_dram_parameter("kb", [T], F32, isOutput=False)
    out = nc.declare_dram_parameter("out", [T, D], BF16, isOutput=True)

    NT = T // 512   # 4 T-ranges for projections
    NK = T // 128   # 16 key tiles

    with tile.TileContext(nc) as tc:
        with (
            tc.tile_pool(name="const", bufs=1) as cpool,
            tc.tile_pool(name="xw", bufs=1) as xwpool,
            tc.tile_pool(name="qk", bufs=1) as qkpool,
            tc.tile_pool(name="raw", bufs=3) as rawpool,
            tc.tile_pool(name="p", bufs=6) as ppool,
            tc.tile_pool(name="rec", bufs=2) as rpool,
            tc.tile_pool(name="ev", bufs=3) as evpool,
            tc.tile_pool(name="psP", bufs=2, space="PSUM") as psP,
            tc.tile_pool(name="psS", bufs=2, space="PSUM") as psS,
            tc.tile_pool(name="psY", bufs=1, space="PSUM") as psY,
        ):
            # ---- constant / weight loads ----
            # order matters: wq + the first xT column-chunks gate the first
            # matmul groups, so issue them first; xT is loaded in 512-col
            # chunks so projection groups start after ~1MB, not 4MB.
            wq_sb = xwpool.tile([128, 8, HD], BF16, tag="wq")
            wk_sb = xwpool.tile([128, 8, HD], BF16, tag="wk")
            wv_sb = xwpool.tile([128, 8, HD], BF16, tag="wv")
            wo_sb = xwpool.tile([128, 2, D], BF16, tag="wo")
            brow_sb = cpool.tile([1, 2 * HD], BF16, tag="brow")
            ones_sb = cpool.tile([1, 512], BF16, tag="ones")
            bv_sb = cpool.tile([128, HD], F32, tag="bv")
            wq_r = wq.ap().rearrange("(c p) n -> p c n", p=128)
            wk_r = wk.ap().rearrange("(c p) n -> p c n", p=128)
            # pair-0 (c2=0) weight halves first: the first attention quarter
            # needs only these
            nc.sync.dma_start(wq_sb[:, :, 0:128], wq_r[:, :, 0:128])
            nc.sync.dma_start(brow_sb[:], brow.ap())
            nc.vector.memset(ones_sb[:], 1.0)

            xts = []
            for dc in range(8):
                xt = xwpool.tile([128, T], BF16, tag=f"xt{dc}", name=f"xt{dc}")
                xts.append(xt)

            def load_xt_tr(tr):
                for dc in range(8):
                    nc.sync.dma_start(
                        xts[dc][:, tr * 512:(tr + 1) * 512],
                        xT[dc * 128:(dc + 1) * 128, tr * 512:(tr + 1) * 512],
                    )

            cos_sb = cpool.tile([128, T], BF16, tag="cos")
            sin_sb = cpool.tile([128, T], BF16, tag="sin")
            tri2_sb = cpool.tile([128, 2, 128], BF16, tag="tri2")
            kb_sb = cpool.tile([128, NK], F32, tag="kb")
            # everything the first attention quarter needs (q/k tr0 chunks,
            # rope tables, wv, exp consts) comes before the bulk of xT so
            # the lead-in isn't DMA-gated
            load_xt_tr(0)
            nc.sync.dma_start(wk_sb[:, :, 0:128], wk_r[:, :, 0:128])
            nc.sync.dma_start(cos_sb[:, 0:512], cos2[:, 0:512])
            nc.sync.dma_start(sin_sb[:, 0:512], sin2[:, 0:512])
            nc.sync.dma_start(wv_sb[:], wv.ap().rearrange("(c p) n -> p c n", p=128))
            nc.sync.dma_start(
                tri2_sb[:], tri2.ap().rearrange("p (two q) -> p two q", two=2))
            nc.sync.dma_start(kb_sb[:], kb.ap().rearrange("(t p) -> p t", p=128))
            nc.sync.dma_start(bv_sb[:], bv.ap().to_broadcast((128, HD)))
            load_xt_tr(1)
            nc.sync.dma_start(cos_sb[:, 512:T], cos2[:, 512:T])
            nc.sync.dma_start(sin_sb[:, 512:T], sin2[:, 512:T])
            load_xt_tr(2)
            load_xt_tr(3)
            nc.sync.dma_start(wq_sb[:, :, 128:256], wq_r[:, :, 128:256])
            nc.sync.dma_start(wk_sb[:, :, 128:256], wk_r[:, :, 128:256])
            nc.sync.dma_start(wo_sb[:], wo.ap().rearrange("(c p) n -> p c n", p=128))

            # persistent [128, T] tiles: 2 heads each (rows 0:64 / 64:128)
            qT = [qkpool.tile([128, T], BF16, tag=f"qT{c}", name=f"qT{c}") for c in range(2)]
            kT = [qkpool.tile([128, T], BF16, tag=f"kT{c}", name=f"kT{c}") for c in range(2)]
            yT = [qkpool.tile([128, T], BF16, tag=f"yT{c}", name=f"yT{c}") for c in range(2)]

            # ---- Q^T / K^T projection + RoPE, one 512-col chunk ----
            # ti: 0 = q, 1 = k (selects bias row slice)
            def proj_qk_chunk(ti, wsb, c2, fin, tr):
                lo, hi = tr * 512, (tr + 1) * 512
                ps = psP.tile([128, 512], F32, tag="pp")
                for dc in range(8):
                    nc.tensor.matmul(
                        ps[:],
                        wsb[:, dc, c2 * 128:(c2 + 1) * 128],
                        xts[dc][:, lo:hi],
                        start=(dc == 0),
                        stop=False,
                    )
                # bias via rank-1 update: ones row x bias row
                nc.tensor.matmul(
                    ps[:],
                    brow_sb[0:1, ti * HD + c2 * 128: ti * HD + (c2 + 1) * 128],
                    ones_sb[0:1, :],
                    start=False, stop=True,
                )
                raw = rawpool.tile([128, 512], BF16, tag="raw")
                nc.vector.tensor_copy(raw[:], ps[:])
                # RoPE: fin = raw*cos + rot(raw)*sin_signed
                f = fin
                for (do, di) in ((0, 32), (32, 0), (64, 96), (96, 64)):
                    nc.vector.tensor_copy(f[do:do + 32, lo:hi], raw[di:di + 32, :])
                nc.vector.tensor_mul(f[:, lo:hi], f[:, lo:hi], sin_sb[:, lo:hi])
                nc.vector.tensor_mul(raw[:], raw[:], cos_sb[:, lo:hi])
                # final add on GpSimd (idle) to unload DVE
                nc.gpsimd.tensor_add(f[:, lo:hi], f[:, lo:hi], raw[:])

            # same projection for TWO 512-col chunks, dc-major over two psum
            # tiles: each stationary slice serves both chunks back-to-back so
            # _dedup_ldweights deletes every second (identical) weight load.
            def proj_qk_pair(ti, wsb, c2, fin, trp):
                trs = (2 * trp, 2 * trp + 1)
                pss = [psP.tile([128, 512], F32, tag="pp", name=f"pp{t}")
                       for t in range(2)]
                for dc in range(8):
                    for t in range(2):
                        lo = trs[t] * 512
                        nc.tensor.matmul(
                            pss[t][:],
                            wsb[:, dc, c2 * 128:(c2 + 1) * 128],
                            xts[dc][:, lo:lo + 512],
                            start=(dc == 0),
                            stop=False,
                            skip_group_check=True,
                        )
                for t in range(2):
                    nc.tensor.matmul(
                        pss[t][:],
                        brow_sb[0:1, ti * HD + c2 * 128: ti * HD + (c2 + 1) * 128],
                        ones_sb[0:1, :],
                        start=False, stop=True,
                        skip_group_check=True,
                    )
                for t in range(2):
                    lo = trs[t] * 512
                    hi = lo + 512
                    raw = rawpool.tile([128, 512], BF16, tag="raw")
                    nc.vector.tensor_copy(raw[:], pss[t][:])
                    f = fin
                    for (do, di) in ((0, 32), (32, 0), (64, 96), (96, 64)):
                        nc.vector.tensor_copy(f[do:do + 32, lo:hi], raw[di:di + 32, :])
                    nc.vector.tensor_mul(f[:, lo:hi], f[:, lo:hi], sin_sb[:, lo:hi])
                    nc.vector.tensor_mul(raw[:], raw[:], cos_sb[:, lo:hi])
                    nc.gpsimd.tensor_add(f[:, lo:hi], f[:, lo:hi], raw[:])

            # ---- V projection (normal layout, with bias and ones blocks) ----
            vts = [None] * NK

            def proj_v(kt):
                ps = psP.tile([128, HD], F32, tag="pp")
                for dc in range(8):
                    nc.tensor.matmul(
                        ps[:],
                        xts[dc][:, kt * 128:(kt + 1) * 128],
                        wv_sb[:, dc, :],
                        start=(dc == 0),
                        stop=(dc == 7),
                    )
                vt = xwpool.tile([128, HG, 128], BF16, tag=f"v{kt}", name=f"v{kt}")
                nc.vector.tensor_add(
                    vt[:, :, 0:64],
                    ps[:].rearrange("p (h d) -> p h d", h=HG),
                    bv_sb[:].rearrange("p (h d) -> p h d", h=HG),
                )
                nc.vector.memset(vt[:, :, 64:128], 1.0)
                vts[kt] = vt

            # ---- attention for head pair c2, one q-quarter (512 cols) ----
            # scores/probs/Y for both heads live side by side in one
            # [128, 2, 512] tile: [:, 0, :] = head 2*c2, [:, 1, :] = 2*c2+1.
            def attn_quarter(c2, qq):
                qlo = qq * 512
                last = 4 * qq + 3
                y = psY.tile([128, 2, 512], F32, tag="y", name=f"y{c2}_{qq}")
                for kt in range(last + 1):
                    j = kt - 4 * qq
                    c = j * 128 if j >= 0 else 0   # first valid col (diag trim)
                    ksl = slice(kt * 128, (kt + 1) * 128)
                    qsl = slice(qlo + c, qlo + 512)
                    s = psS.tile([128, 2, 512], F32, tag="s")
                    # two row-group-concurrent 64-row score matmuls
                    nc.tensor.matmul(
                        s[:, 0, c:], kT[c2][0:64, ksl], qT[c2][0:64, qsl],
                        start=True, stop=True,
                    )
                    nc.tensor.matmul(
                        s[:, 1, c:], kT[c2][64:128, ksl], qT[c2][64:128, qsl],
                        start=True, stop=True,
                    )
                    p = ppool.tile([128, 2, 512], BF16, tag="p")
                    nc.scalar.activation(
                        p[:, :, c:], s[:, :, c:],
                        mybir.ActivationFunctionType.Exp,
                        bias=kb_sb[:, kt:kt + 1], scale=SCALE,
                    )
                    if j >= 0:
                        # diagonal 128x128 blocks of both heads: tri mask
                        nc.vector.tensor_mul(
                            p[:, :, c:c + 128], p[:, :, c:c + 128], tri2_sb[:]
                        )
                    for h in (0, 1):
                        nc.tensor.matmul(
                            y[:, h, c:],
                            vts[kt][:, 2 * c2 + h, :],
                            p[:, h, c:],
                            start=(kt == 0),
                            stop=(kt == last),
                            skip_group_check=True,
                        )
                # normalize both heads at once: 1/r = exp(-ln r)
                lnr = rpool.tile([64, 2, 512], F32, tag="lnr")
                rec = rpool.tile([64, 2, 512], F32, tag="rec")
                nc.scalar.activation(
                    lnr[:], y[64:128, :, :], mybir.ActivationFunctionType.Ln)
                nc.scalar.activation(
                    rec[:], lnr[:], mybir.ActivationFunctionType.Exp,
                    scale=-1.0)
                nc.vector.tensor_mul(
                    yT[c2][0:64, qlo:qlo + 512], y[0:64, 0, :], rec[:, 0, :])
                nc.vector.tensor_mul(
                    yT[c2][64:128, qlo:qlo + 512], y[0:64, 1, :], rec[:, 1, :])

            def outproj_quarter(qq):
                # partial out for T-tiles of this quarter; host adds bo+reduces
                # c2-outer / dr-inner: each yT stationary slice serves both wo
                # column halves (second load deleted by _dedup_ldweights).
                # Last two quarters evict on ScalarE: the exp stream is done
                # by then, and it shortens the DVE-bound tail.
                on_scalar = qq >= 2
                for tt in range(4 * qq, 4 * qq + 4):
                    pss = [psP.tile([128, 512], F32, tag="pp", name=f"po{t}")
                           for t in range(2)]
                    for c2 in range(2):
                        for dr in range(2):
                            nc.tensor.matmul(
                                pss[dr][:],
                                yT[c2][:, tt * 128:(tt + 1) * 128],
                                wo_sb[:, c2, dr * 512:(dr + 1) * 512],
                                start=(c2 == 0),
                                stop=(c2 == 1),
                                skip_group_check=True,
                            )
                    for dr in range(2):
                        ev = evpool.tile([128, 512], BF16, tag="ev")
                        if on_scalar:
                            nc.scalar.activation(
                                ev[:], pss[dr][:],
                                mybir.ActivationFunctionType.Identity)
                        else:
                            nc.vector.tensor_copy(ev[:], pss[dr][:])
                        nc.sync.dma_start(
                            out[tt * 128:(tt + 1) * 128, dr * 512:(dr + 1) * 512],
                            ev[:],
                        )

            # ---- emission order == scheduler priority ----
            # quarter qq of pair 0 needs exactly q/k chunks tr<=qq and
            # V tiles kt<=4qq+3, so interleave per quarter: attention
            # starts right after the first 1.5MB of DMA instead of after
            # the whole projection phase.
            # quarters 0/1 use single chunks (tr0 must not wait on tr1's xT
            # DMA in the lead-in); later projections go in reuse-pairs
            for qq in range(2):
                proj_qk_chunk(0, wq_sb, 0, qT[0], qq)
                proj_qk_chunk(1, wk_sb, 0, kT[0], qq)
                for kt in range(4 * qq, 4 * qq + 4):
                    proj_v(kt)
                attn_quarter(0, qq)
            proj_qk_pair(0, wq_sb, 0, qT[0], 1)
            proj_qk_pair(1, wk_sb, 0, kT[0], 1)
            for kt in range(8, 12):
                proj_v(kt)
            attn_quarter(0, 2)
            # V 12-15 before the pair-1 projections: attn(0,3) needs them
            # and the psP ring serves groups in emission order
            for kt in range(12, 16):
                proj_v(kt)
            attn_quarter(0, 3)
            # pair-1 tr0/tr1 projections run during attn(0,3) + transition;
            # tr2/tr3 move into the pair-1 window (which has PE slack) so
            # they stop crowding the pair-0 crunch region.  outproj lands
            # one quarter late so its matmuls fill exp-wait gaps instead of
            # outranking the next quarter's score matmuls.
            proj_qk_pair(0, wq_sb, 1, qT[1], 0)
            proj_qk_pair(1, wk_sb, 1, kT[1], 0)
            attn_quarter(1, 0)
            proj_qk_pair(0, wq_sb, 1, qT[1], 1)
            attn_quarter(1, 1)
            proj_qk_pair(1, wk_sb, 1, kT[1], 1)
            outproj_quarter(0)
            attn_quarter(1, 2)
            outproj_quarter(1)
            attn_quarter(1, 3)
            outproj_quarter(2)
            outproj_quarter(3)
    nd = _dedup_ldweights(nc)
    _split_multi_waits(nc)
    assert nd > 0, f"expected ldweights dedup to fire, got {nd}"
    return nc


def _rope_tables():
    inv_freq = 1.0 / (THETA ** (np.arange(0, HS, 2, dtype=np.float64) / HS))  # [32]
    t = np.arange(T, dtype=np.float64)
    fr = t[:, None] * inv_freq[None, :]          # [T, 32]
    emb = np.concatenate([fr, fr], axis=1)       # [T, 64]
    cos = np.cos(emb).T.astype(np.float32)       # [64, T]
    sin = np.sin(emb).T.astype(np.float32)       # [64, T]
    sin_signed = sin.copy()
    sin_signed[0:32] = -sin_signed[0:32]
    cos2 = np.concatenate([cos, cos], axis=0)            # [128, T]
    sin2 = np.concatenate([sin_signed, sin_signed], 0)   # [128, T]
    return cos2.astype(ml_dtypes.bfloat16), sin2.astype(ml_dtypes.bfloat16)


def _in_maps(x, attention_mask, Wq, bqv, Wk, bkv, Wv, bvv, Wo):
    cos2, sin2 = _rope_tables()
    tri = np.triu(np.ones((128, 128), np.float32))
    tri2 = np.concatenate([tri, tri], axis=1).astype(ml_dtypes.bfloat16)
    bf = ml_dtypes.bfloat16
    xTs = [np.ascontiguousarray(x[b].T).astype(bf) for b in range(B)]
    kbs = [
        np.where(attention_mask[b] != 0, 0.0, NEG).astype(np.float32)
        for b in range(B)
    ]
    maps = []
    for core in range(NCORES):
        b, g = core // 4, core % 4
        sl = slice(g * HD, (g + 1) * HD)
        brow = np.concatenate([bqv[sl], bkv[sl]]).reshape(1, 2 * HD)
        maps.append({
            "xT": xTs[b],
            "wq": np.ascontiguousarray(Wq[:, sl]).astype(bf),
            "wk": np.ascontiguousarray(Wk[:, sl]).astype(bf),
            "wv": np.ascontiguousarray(Wv[:, sl]).astype(bf),
            "wo": np.ascontiguousarray(Wo[sl, :]).astype(bf),
            "brow": brow.astype(bf),
            "bv": bvv[sl].astype(np.float32).reshape(1, HD),
            "cos2": cos2,
            "sin2": sin2,
            "tri2": tri2,
            "kb": kbs[b],
        })
    return maps


def _run(inputs, trace=False):
    global _NC
    if _NC is None:
        _NC = build_nc()
    maps = _in_maps(
        np.asarray(inputs["x"]), np.asarray(inputs["attention_mask"]),
        np.asarray(inputs["Wq"]), np.asarray(inputs["bq"]),
        np.asarray(inputs["Wk"]), np.asarray(inputs["bk"]),
        np.asarray(inputs["Wv"]), np.asarray(inputs["bv"]),
        np.asarray(inputs["Wo"]),
    )
    res = run_bass_kernel_spmd(_NC, maps, core_ids=list(range(NCORES)), trace=trace)
    bo = np.asarray(inputs["bo"], np.float32)
    outs = []
    for b in range(B):
        acc = np.zeros((T, D), np.float32)
        for g in range(4):
            acc += np.asarray(res.results[b * 4 + g]["out"], np.float32)
        outs.append(acc + bo[None, :])
    return np.stack(outs, axis=0), res


def kernel(**inputs):
    out, _ = _run(inputs, trace=False)
    return out
